# revision 48
# baseline (speedup 1.0000x reference)
"""Trainium2 Bass kernel: causal multi-head self-attention with RoPE.

Problem: B=4, S=2048, D=1024, H=16 heads, dk=64, fp32.
Sharding: 8 cores = (batch b in 0..3) x (head-group g in 0..1, 8 heads each).
Each core computes a partial o_proj output [S, D] for its (b, g); the host
sums the two head-group partials per batch and stacks batches.

Design (vs the 383us fp32r baseline; this version sims at ~261us):
 - Everything downstream of HBM runs on bf16 operands (x, all weights, q/k
   post-RoPE, exp-scores, v, o) with fp32 psum accumulation: bf16 matmuls
   run at 1 cycle/row at ANY moving size, unlike fp32r which pays 4x under
   moving=256, enabling the flipped AV orientation below.  End-to-end L2
   err ~6e-3 (gate 2e-2).
 - AV matmuls are flipped to out[i-block 128 part, dk moving]: lhsT =
   exp-scores tile pt[j, i-block], rhs = v[j, dk].  Cost is charged per
   moving row, so this halves AV PE time vs the [dk+1 part, i moving]
   orientation (full 128-partition output occupancy).  A parallel 1-col
   matmul against a ones vector accumulates the softmax denominators into a
   separate psum bank, making them per-PARTITION scalars; normalize is then
   reciprocal [128,2] + tensor_scalar multiply, and oT for o_proj comes
   from [128,128] PE transposes (53ns each in bf16) into per-(pair,i-block)
   contiguous SBUF lhsT tiles.
 - Scores for both heads of a pair land in one [128,1024] 2-bank psum tile
   and a single bank-spanning exp converts them (185ns/instr access latency
   amortized; deep-diagonal slots split in two to skip the dead gap).
 - PSUM banks (8): 2x scores [128,1024] + 1x AV [128,512] + 1x denom
   [128,32] + 1x transpose + 1x proj/oproj [128,512] (+padding).  Multiple
   accumulation groups share the AV/denom banks with start=True only on the
   bank's first matmul (the backend's pending-zero region is bank-wide;
   later groups' first touch overwrites, then accumulates).
 - RoPE partner values (partition p^32) come from partition-base-shifted
   DVE tensor_copies (engines allow differing in/out partition bases; bf16
   SBUF copies run in 4x DVE mode), replacing 128 SBUF->SBUF DMAs.
 - Weights/x load as single 3-dim-AP DMAs (e.g. [128, 8dt, cols]) instead
   of per-dt strips: ~60 DMA instructions total vs 247 (HWDGE issue and
   SP.SEQ are ~630ns/DMA serial resources).  wq/wk load their cc0 column
   slice and the first 512 cos/sin columns first so projection + RoPE for
   the first attention slot are ready ~6us in.
 - The emission interleaves attention slots (pair, j-tile) with side work:
   projections of later s-tiles (q two tiles ahead, k/v one - k/v of
   s-tile st are only needed from j-tile 4*st of attention tile st) and
   o_proj of the previous tile, paced evenly across each tile's slots and
   placed between the scores issue and the exp-blocked AV block (the PE's
   4-deep wait queue can't bypass a clogged dependent block).  Engine
   assignment of psum-draining copies alternates DVE/ACT by phase to track
   which engine has slack (DVE is the early bottleneck, ACT's exp the late
   one).
"""

import numpy as np
import ml_dtypes
from contextlib import ExitStack

import concourse.bass as bass
import concourse.bacc as bacc
import concourse.mybir as mybir
import concourse.tile as tile
from concourse.ap import AP
from concourse.bass_utils import run_bass_kernel_spmd

B, S, D = 4, 2048, 1024
H_TOT, DK = 16, 64
THETA = 10000.0
N_CORES = 8
HG = 2                  # head groups (cores per batch)
H_LOC = H_TOT // HG     # 8 heads per core
CL = H_LOC * DK         # 512 local channels
P = 128
DT = D // P             # 8 contraction tiles
CC = CL // P            # 4 c-chunks (2 heads each)
ST = S // 512           # 4 s-tiles of 512
F32 = mybir.dt.float32
F32R = mybir.dt.float32r
BF16 = mybir.dt.bfloat16
OP = mybir.AluOpType
AF = mybir.ActivationFunctionType

_CACHE: dict = {}
LAST_RESULT = None  # stashed BassKernelResults for test harness introspection


def _build_program():
    nc = bacc.Bacc(
        "TRN2", target_bir_lowering=False, debug=False,
        num_devices=N_CORES,
    )
    xT = nc.declare_dram_parameter("xT", [D, S], BF16, isOutput=False).ap()
    wq = nc.declare_dram_parameter("wq", [D, CL], BF16, isOutput=False).ap()
    wk = nc.declare_dram_parameter("wk", [D, CL], BF16, isOutput=False).ap()
    wv = nc.declare_dram_parameter("wv", [D, CL], BF16, isOutput=False).ap()
    wo = nc.declare_dram_parameter("wo", [CL, D], BF16, isOutput=False).ap()
    cosr = nc.declare_dram_parameter("cosr", [P, S], BF16, isOutput=False).ap()
    sinr = nc.declare_dram_parameter("sinr", [P, S], BF16, isOutput=False).ap()
    msk = nc.declare_dram_parameter("msk", [P, P], BF16, isOutput=False).ap()
    ones1 = nc.declare_dram_parameter("ones1", [P, 1], BF16, isOutput=False).ap()
    ident = nc.declare_dram_parameter("ident", [P, P], BF16, isOutput=False).ap()
    out = nc.declare_dram_parameter("out", [S, D], F32, isOutput=True).ap()

    with tile.TileContext(nc) as tc:
        with ExitStack() as ctx, nc.allow_low_precision(
                reason="bf16 attention operands; psum accumulation stays fp32"):
            _emit(nc, tc, ctx, xT, wq, wk, wv, wo, cosr, sinr, msk, ones1, ident, out)
    nc.finalize()
    return nc


def _emit(nc, tc, ctx, xT, wq, wk, wv, wo, cosr, sinr, msk, ones1, ident, out):
    consts = ctx.enter_context(tc.tile_pool(name="consts", bufs=1))
    kv_pool = ctx.enter_context(tc.tile_pool(name="kv", bufs=1))
    xt_pool = ctx.enter_context(tc.tile_pool(name="xt", bufs=3))
    qt_pool = ctx.enter_context(tc.tile_pool(name="qt", bufs=3))
    rope_pool = ctx.enter_context(tc.tile_pool(name="rope", bufs=2))
    pt_pool = ctx.enter_context(tc.tile_pool(name="pt", bufs=3))
    on_pool = ctx.enter_context(tc.tile_pool(name="on", bufs=3))
    rec_pool = ctx.enter_context(tc.tile_pool(name="rec", bufs=2))
    ot_pool = ctx.enter_context(tc.tile_pool(name="ot", bufs=3))
    ob_pool = ctx.enter_context(tc.tile_pool(name="ob", bufs=2))
    sc_ps = ctx.enter_context(tc.tile_pool(name="scps", bufs=2, space="PSUM"))
    av_ps = ctx.enter_context(tc.tile_pool(name="avps", bufs=1, space="PSUM"))
    ad_ps = ctx.enter_context(tc.tile_pool(name="adps", bufs=1, space="PSUM"))
    tr_ps = ctx.enter_context(tc.tile_pool(name="trps", bufs=1, space="PSUM"))
    pp_ps = ctx.enter_context(tc.tile_pool(name="pp", bufs=1, space="PSUM"))

    # ---- resident constants --------------------------------------------
    wq_sb = consts.tile([P, DT, CL], BF16, name="wq_sb")
    wk_sb = consts.tile([P, DT, CL], BF16, name="wk_sb")
    wv_sb = consts.tile([P, DT, CL], BF16, name="wv_sb")
    wo_sb = consts.tile([P, CC, D], BF16, name="wo_sb")
    cos_sb = consts.tile([P, S], BF16, name="cos_sb")
    sin_sb = consts.tile([P, S], BF16, name="sin_sb")
    msk_sb = consts.tile([P, P], BF16, name="msk_sb")
    ones_sb = consts.tile([P, 1], BF16, name="ones_sb")
    id_sb = consts.tile([P, P], BF16, name="id_sb")
    kT_t = [kv_pool.tile([P, CC, 512], BF16, name=f"kT{st}") for st in range(ST)]
    vv_t = [kv_pool.tile([P, H_LOC, DK], BF16, name=f"vv{j}")
            for j in range(S // P)]

    def w_slice_dma(dst_sb, w_ap, c0, c1):
        # dst_sb[:, dt, c0:c1] <- w[dt*128+p, c0:c1] in ONE 3-dim DMA
        src = AP(w_ap.tensor, w_ap.offset + c0, [[CL, P], [CL * P, DT], [1, c1 - c0]])
        nc.sync.dma_start(dst_sb[:, :, c0:c1], src)

    def load_consts():
        # ordered for fast compute start: first proj group is q(cc0, st0)
        w_slice_dma(wq_sb, wq, 0, P)
        for dt in range(DT):  # xt st0 per-dt so accumulation can chase arrivals
            nc.sync.dma_start(
                xt0[:, dt, :], xT[dt * P:(dt + 1) * P, 0:512])
        w_slice_dma(wk_sb, wk, 0, P)
        # first 512 cols of cos/sin unblock st0's RoPE ~5us earlier
        nc.sync.dma_start(cos_sb[:, 0:512], cosr[:, 0:512])
        nc.sync.dma_start(sin_sb[:, 0:512], sinr[:, 0:512])
        for dt in range(DT):  # per-dt so v0's accumulation chases arrivals
            nc.sync.dma_start(
                wv_sb[:, dt, :], wv[dt * P:(dt + 1) * P, :])
        nc.sync.dma_start(cos_sb[:, 512:S], cosr[:, 512:S])
        nc.sync.dma_start(sin_sb[:, 512:S], sinr[:, 512:S])
        nc.sync.dma_start(msk_sb[:], msk)
        nc.sync.dma_start(ones_sb[:], ones1)
        nc.sync.dma_start(id_sb[:], ident)
        w_slice_dma(wq_sb, wq, P, CL)
        w_slice_dma(wk_sb, wk, P, CL)
        wo_src = AP(wo.tensor, wo.offset, [[D, P], [D * P, CC], [1, D]])
        nc.sync.dma_start(wo_sb[:], wo_src)

    from collections import deque

    def load_st(st):
        s0 = st * 512
        xt = xt_pool.tile([P, DT, 512], BF16, name="xt")
        src = AP(xT.tensor, xT.offset + s0, [[S, P], [S * P, DT], [1, 512]])
        nc.sync.dma_start(xt[:], src)
        qTs = qt_pool.tile([P, CC, 512], BF16, name="qTs")
        return xt, qTs

    def qk_group(st, cc, which, ctx_tiles):
        xt, qTs = ctx_tiles
        s0 = st * 512
        w_sb = wq_sb if which == "q" else wk_sb
        dst = qTs[:, cc, :] if which == "q" else kT_t[st][:, cc, :]
        ps = pp_ps.tile([P, 512], F32, name="pp")
        for dt in range(DT):
            nc.tensor.matmul(
                ps[:], lhsT=(w_sb[:, dt, cc * P:(cc + 1) * P]),
                rhs=(xt[:, dt, :]), start=(dt == 0), stop=(dt == DT - 1))
        qraw = rope_pool.tile([P, 512], BF16, name="qraw")
        # psum->sbuf copy on ACT for early s-tiles (ACT idles while DVE is
        # the early bottleneck); DVE for late ones (ACT saturates on exp)
        if st <= 1:
            nc.scalar.copy(qraw[:], ps[:])
        else:
            nc.vector.tensor_copy(qraw[:], ps[:])
        # partner view (partition p ^ 32) via partition-base-shifted DVE
        # copies (bf16 SBUF copies run in 4x DVE mode)
        qsh = rope_pool.tile([P, 512], BF16, name="qsh")
        for blk in range(4):
            p0 = blk * 32
            q0 = p0 ^ 32
            nc.vector.tensor_copy(qsh[p0:p0 + 32, :], qraw[q0:q0 + 32, :])
        tcos = rope_pool.tile([P, 512], BF16, name="tcos")
        nc.gpsimd.tensor_tensor(tcos[:], qraw[:], cos_sb[:, s0:s0 + 512], OP.mult)
        nc.vector.tensor_tensor(dst, qsh[:], sin_sb[:, s0:s0 + 512], OP.mult)
        nc.vector.tensor_tensor(dst, dst, tcos[:], OP.add)

    def v_group(st, sc, ctx_tiles):
        xt = ctx_tiles[0]
        gsc = st * 4 + sc
        ps = pp_ps.tile([P, 512], F32, name="pp")
        for dt in range(DT):
            nc.tensor.matmul(
                ps[:], lhsT=(xt[:, dt, sc * P:(sc + 1) * P]),
                rhs=(wv_sb[:, dt, :]), start=(dt == 0), stop=(dt == DT - 1))
        if st <= 1:
            nc.scalar.copy(
                vv_t[gsc][:], ps.rearrange("p (h c) -> p h c", c=DK))
        else:
            nc.vector.tensor_copy(
                vv_t[gsc][:], ps.rearrange("p (h c) -> p h c", c=DK))

    def oproj_group(ti, oT_blks, et, sc4, use_sc_pool=False):
        s0 = ti * 512
        if use_sc_pool:
            ps = sc_ps.tile([P, 512], F32, name="sc")
        else:
            ps = pp_ps.tile([P, 512], F32, name="pp")
        for cc in range(CC):
            nc.tensor.matmul(
                ps[:], lhsT=(oT_blks[cc][sc4][:]),
                rhs=(wo_sb[:, cc, et * 512:(et + 1) * 512]),
                start=(cc == 0), stop=(cc == CC - 1))
        osb = ob_pool.tile([P, 512], F32, name="osb")
        # psum->sbuf staging on DVE (ACT stays exp-only; GPSIMD cannot
        # read PSUM)
        nc.vector.tensor_copy(osb[:], ps[:])
        nc.sync.dma_start(
            out[s0 + sc4 * P:s0 + (sc4 + 1) * P,
                et * 512:(et + 1) * 512], osb[:])

    # prologue: minimal inline projections (q-cc0, k-cc0, v0); the rest of
    # st0's projections go through the global side queue so attention(ti=0)
    # starts ~7us in
    xt0 = xt_pool.tile([P, DT, 512], BF16, name="xt")
    qTs0 = qt_pool.tile([P, CC, 512], BF16, name="qTs")
    ctx0 = (xt0, qTs0)
    load_consts()
    qk_group(0, 0, "q", ctx0)
    qk_group(0, 0, "k", ctx0)
    v_group(0, 0, ctx0)
    st_tiles = {0: ctx0}

    # ------------------------------------------------------------------
    # Global side-work queue.  k/v projections of s-tile st are legal to
    # run DURING attention(ti=st) up to jt=4*st; q projections of st must
    # finish before ti=st starts.  o_proj(ti) runs during ti+1.
    # ------------------------------------------------------------------
    side = deque()
    deferred_oproj = None

    def seed_side_for_ti(ti):
        # called at ti start: queue o_proj(ti-1), then k/v(st=ti) early-need
        # groups were queued at ti-1... layout below.
        pass

    prev_oproj = None
    for ti in range(ST):
        qTs = st_tiles[ti][1]
        # deferred k/v of THIS s-tile first (needed from jt=4*ti onward),
        # then o_proj of the previous tile, then next tile's q projections.
        if ti >= 1:
            ctxn = st_tiles[ti]
            side.appendleft((qk_group, ti, 0, "k", ctxn))
            pos = 1
            for sc in range(4):
                side.insert(pos, (v_group, ti, sc, ctxn))
                pos += 1
            for cc in range(1, CC):
                side.insert(pos, (qk_group, ti, cc, "k", ctxn))
                pos += 1
        if deferred_oproj is not None:
            pti, poT = deferred_oproj
            for sc4 in range(4):
                side.append((oproj_group, pti, poT, 1, sc4))
            deferred_oproj = None
        if prev_oproj is not None:
            pti, poT = prev_oproj
            for et in range(2):
                for sc4 in range(4):
                    if pti == 1 and et == 1:
                        continue  # deferred into ti3
                    side.append((oproj_group, pti, poT, et, sc4))
            if pti == 1:
                deferred_oproj = (pti, poT)
            prev_oproj = None
        if ti + 1 < ST and ti + 1 not in st_tiles:
            nxt = load_st(ti + 1)
            st_tiles[ti + 1] = nxt
            if ti + 1 == 1:
                # remaining st0 groups first (attention ti0 needs them)
                side.extend([(v_group, 0, sc, ctx0) for sc in (1, 2, 3)])
                side.extend([(qk_group, 0, cc, w, ctx0)
                             for cc in (1, 2, 3) for w in ("q", "k")])
            if ti == 0:
                # q projections of st1 (must complete before ti1)
                side.extend([(qk_group, ti + 1, cc, "q", nxt)
                             for cc in range(CC)])
        if ti + 2 < ST and ti + 2 not in st_tiles:
            nxt2 = load_st(ti + 2)
            st_tiles[ti + 2] = nxt2
            # q projections of st+2 run in THIS tile's tail (they only need
            # xt(st+2), an input) so ti+1 keeps its slots for k/v + o_proj
            side.extend([(qk_group, ti + 2, cc, "q", nxt2)
                         for cc in range(CC)])

        oT_blks = [[ot_pool.tile([P, P], BF16, name=f"oTb_{pr}_{b}")
                    for b in range(4)] for pr in range(CC)]
        ad = ad_ps.tile([P, 32], F32, name="ad")
        njt = 4 * (ti + 1)
        slots = [(pair, jt) for pair in range(CC) for jt in range(njt)]
        state = {"ad_started": False, "av": None, "av_started": False,
                 "rec": None, "pt": {}, "sc": {}}

        def phase1(slot):
            pair, jt = slot
            r = max(0, (jt - 4 * ti) * P)
            kslc = kT_t[jt // 4][:, pair, (jt % 4) * P:(jt % 4) * P + P]
            scb = sc_ps.tile([P, 1024], F32, name="sc")
            nc.tensor.matmul(
                scb[:, r:512], lhsT=(kslc[0:DK]),
                rhs=(qTs[0:DK, pair, r:512]), start=True, stop=True)
            nc.tensor.matmul(
                scb[:, 512 + r:1024], lhsT=(kslc[DK:P]),
                rhs=(qTs[DK:P, pair, r:512]), start=True, stop=True,
                skip_group_check=True)
            pt = pt_pool.tile([P, 1024], BF16, name="pt")
            if r >= 256:
                # deep-diagonal slots: two exps beat paying for the dead gap
                nc.scalar.activation(
                    pt[:, r:512], scb[:, r:512], AF.Exp, scale=0.125)
                nc.scalar.activation(
                    pt[:, 512 + r:1024], scb[:, 512 + r:1024], AF.Exp,
                    scale=0.125)
            else:
                # one bank-spanning exp covering both heads' ranges; the gap
                # cols [512:512+r) hold stale psum whose exp is never read
                nc.scalar.activation(
                    pt[:, r:1024], scb[:, r:1024], AF.Exp, scale=0.125)
            state["sc"][slot] = scb
            if jt >= 4 * ti:
                nc.vector.tensor_tensor(
                    pt[:, r:r + P], pt[:, r:r + P], msk_sb[:], OP.mult)
                nc.vector.tensor_tensor(
                    pt[:, 512 + r:512 + r + P], pt[:, 512 + r:512 + r + P],
                    msk_sb[:], OP.mult)
            state["pt"][slot] = pt

        def phase2(slot):
            pair, jt = slot
            pt = state["pt"].pop(slot)
            state["sc"].pop(slot)
            if jt == 0:
                state["av"] = av_ps.tile([P, 512], F32, name="av")
                state["av_started"] = False
                state["rec"] = rec_pool.tile([P, 8], F32, name="rec")
            av, rec = state["av"], state["rec"]
            ib0 = max(0, jt - 4 * ti)
            # h0 AV matmuls first (they only wait on the h0 exp), then h1
            for hh in range(2):
                for ib in range(ib0, 4):
                    last = (jt == 4 * ti + ib)
                    h = 2 * pair + hh
                    lhs = (pt[:, ib * P:ib * P + P] if hh == 0
                           else pt[:, 512 + ib * P:512 + ib * P + P])
                    nc.tensor.matmul(
                        av[:, ib * P + hh * DK:ib * P + hh * DK + DK],
                        lhsT=(lhs), rhs=(vv_t[jt][:, h, :]),
                        start=(not state["av_started"]), stop=last,
                        skip_group_check=True)
                    state["av_started"] = True
                    dc = pair * 8 + ib * 2 + hh
                    nc.tensor.matmul(
                        ad[:, dc:dc + 1], lhsT=(lhs), rhs=(ones_sb[:]),
                        start=(not state["ad_started"]), stop=last,
                        skip_group_check=True)
                    state["ad_started"] = True
            ibd = jt - 4 * ti
            if 0 <= ibd <= 3:
                ib = ibd
                # drain (pair, ib): per-partition normalize, PE transpose,
                # copy out to the SBUF lhsT tile for o_proj
                c2 = pair * 8 + ib * 2
                nc.vector.reciprocal(
                    rec[:, ib * 2:ib * 2 + 2], ad[:, c2:c2 + 2])
                o_n = on_pool.tile([P, P], BF16, name="o_n")
                nc.vector.tensor_scalar(
                    o_n[:, 0:DK],
                    av[:, ib * P:ib * P + DK],
                    rec[:, ib * 2:ib * 2 + 1], None, OP.mult)
                nc.vector.tensor_scalar(
                    o_n[:, DK:2 * DK],
                    av[:, ib * P + DK:ib * P + 2 * DK],
                    rec[:, ib * 2 + 1:ib * 2 + 2], None, OP.mult)
                trp = tr_ps.tile([P, P], BF16, name="tr")
                nc.tensor.matmul(
                    trp[:], lhsT=(o_n[:]), rhs=(id_sb[:]),
                    is_transpose=True, start=True, stop=True,
                    skip_group_check=True)
                nc.vector.tensor_copy(oT_blks[pair][ib][:], trp[:])

        # depth-2 software pipeline: sc/exp of slot i+2 issue before the AV
        # block of slot i, so exp latency hides behind PE work
        # ti3's own o_proj for i-block column sc4 only needs pair3's ib=sc4
        # drain (at slot 60+sc4); pull the early columns into the last slots
        tail_oproj = deque()
        if ti == ST - 1:
            for sc4 in range(4):
                for et in range(2):
                    tail_oproj.append((oproj_group, ti, oT_blks, et, sc4))

        nslots = len(slots)
        side_n = len(side)
        popped = 0
        for i in range(nslots):
            phase1(slots[i])
            # side work lands between the scores/exp issue and the
            # exp-blocked AV block (the PE's 4-deep wait queue means work
            # emitted after phase2 can't bypass the stalled AV matmuls),
            # paced evenly across the tile's slots
            want = -(-side_n * (i + 1) // nslots)  # ceil
            while popped < want and side:
                g = side.popleft()
                g[0](*g[1:])
                popped += 1
            phase2(slots[i])
            if ti == ST - 1 and i >= 61 and tail_oproj:
                for _ in range(1 if i == 61 else 2):
                    if tail_oproj:
                        g = tail_oproj.popleft()
                        g[0](*g[1:])
        while side:
            g = side.popleft()
            g[0](*g[1:])
        prev_oproj = (ti, oT_blks)

    # final o_proj (whatever ti3 didn't absorb): alternate between the pp
    # bank and the now-idle scores banks so the tail pipelines
    use_sc = False
    while tail_oproj:
        g = tail_oproj.popleft()
        g[0](*g[1:], use_sc_pool=use_sc)
        use_sc = not use_sc


def _host_prep(x, Wq, Wk, Wv, Wo, token_positions):
    """Build the 8 per-core input maps (sharding + layout prep only)."""
    x = np.asarray(x, dtype=np.float32)
    Wq = np.asarray(Wq, dtype=np.float32)
    Wk = np.asarray(Wk, dtype=np.float32)
    Wv = np.asarray(Wv, dtype=np.float32)
    Wo = np.asarray(Wo, dtype=np.float32)
    pos = np.asarray(token_positions)

    half = DK // 2
    inv_freq = THETA ** (-np.arange(half, dtype=np.float64) * 2.0 / DK)
    ang = pos.astype(np.float64)[None, :] * inv_freq[:, None]      # [32, S]
    cos32 = np.cos(ang)
    sin32 = np.sin(ang)
    cosr = np.empty((P, S), dtype=np.float32)
    sinr = np.empty((P, S), dtype=np.float32)
    for p in range(P):
        ip = p % DK
        i = ip % half
        cosr[p] = cos32[i]
        sinr[p] = (-sin32[i]) if ip < half else sin32[i]
    cosr = cosr.astype(ml_dtypes.bfloat16)
    sinr = sinr.astype(ml_dtypes.bfloat16)

    # de-interleave permutation within each head: [evens | odds]
    perm = np.concatenate([np.arange(0, DK, 2), np.arange(1, DK, 2)])

    msk = np.triu(np.ones((P, P), dtype=np.float32)).astype(ml_dtypes.bfloat16)
    ones1 = np.ones((P, 1), dtype=np.float32).astype(ml_dtypes.bfloat16)
    ident = np.eye(P, dtype=np.float32).astype(ml_dtypes.bfloat16)

    WqT = Wq.T  # [d_in, e_out]
    WkT = Wk.T
    WvT = Wv.T
    WoT = Wo.T  # [e_in, d_out]

    in_maps = []
    for core in range(N_CORES):
        b, g = core // HG, core % HG
        cols = np.concatenate(
            [g * CL + h * DK + perm for h in range(H_LOC)])
        plain = slice(g * CL, (g + 1) * CL)
        in_maps.append({
            "xT": np.ascontiguousarray(x[b].T).astype(ml_dtypes.bfloat16),
            "wq": np.ascontiguousarray(WqT[:, cols]).astype(ml_dtypes.bfloat16),
            "wk": np.ascontiguousarray(WkT[:, cols]).astype(ml_dtypes.bfloat16),
            "wv": np.ascontiguousarray(WvT[:, plain]).astype(ml_dtypes.bfloat16),
            "wo": np.ascontiguousarray(WoT[plain, :]).astype(ml_dtypes.bfloat16),
            "cosr": cosr,
            "sinr": sinr,
            "msk": msk,
            "ones1": ones1,
            "ident": ident,
        })
    return in_maps


def kernel(x, Wq, Wk, Wv, Wo, token_positions, _trace=False):
    global LAST_RESULT
    if "nc" not in _CACHE:
        _CACHE["nc"] = _build_program()
    nc = _CACHE["nc"]

    in_maps = _host_prep(x, Wq, Wk, Wv, Wo, token_positions)
    res = run_bass_kernel_spmd(nc, in_maps, core_ids=list(range(N_CORES)),
                               trace=_trace)
    LAST_RESULT = res
    outs = [r["out"] for r in res.results]
    final = np.empty((B, S, D), dtype=np.float32)
    for b in range(B):
        final[b] = outs[b * HG]
        for g in range(1, HG):
            final[b] += outs[b * HG + g]
    return final


# revision 49
# speedup vs baseline: 1.0037x; 1.0037x over previous
"""Trainium2 Bass kernel: causal multi-head self-attention with RoPE.

Problem: B=4, S=2048, D=1024, H=16 heads, dk=64, fp32.
Sharding: 8 cores = (batch b in 0..3) x (head-group g in 0..1, 8 heads each).
Each core computes a partial o_proj output [S, D] for its (b, g); the host
sums the two head-group partials per batch and stacks batches.

Design (vs the 383us fp32r baseline; this version sims at ~261us):
 - Everything downstream of HBM runs on bf16 operands (x, all weights, q/k
   post-RoPE, exp-scores, v, o) with fp32 psum accumulation: bf16 matmuls
   run at 1 cycle/row at ANY moving size, unlike fp32r which pays 4x under
   moving=256, enabling the flipped AV orientation below.  End-to-end L2
   err ~6e-3 (gate 2e-2).
 - AV matmuls are flipped to out[i-block 128 part, dk moving]: lhsT =
   exp-scores tile pt[j, i-block], rhs = v[j, dk].  Cost is charged per
   moving row, so this halves AV PE time vs the [dk+1 part, i moving]
   orientation (full 128-partition output occupancy).  A parallel 1-col
   matmul against a ones vector accumulates the softmax denominators into a
   separate psum bank, making them per-PARTITION scalars; normalize is then
   reciprocal [128,2] + tensor_scalar multiply, and oT for o_proj comes
   from [128,128] PE transposes (53ns each in bf16) into per-(pair,i-block)
   contiguous SBUF lhsT tiles.
 - Scores for both heads of a pair land in one [128,1024] 2-bank psum tile
   and a single bank-spanning exp converts them (185ns/instr access latency
   amortized; deep-diagonal slots split in two to skip the dead gap).
 - PSUM banks (8): 2x scores [128,1024] + 1x AV [128,512] + 1x denom
   [128,32] + 1x transpose + 1x proj/oproj [128,512] (+padding).  Multiple
   accumulation groups share the AV/denom banks with start=True only on the
   bank's first matmul (the backend's pending-zero region is bank-wide;
   later groups' first touch overwrites, then accumulates).
 - RoPE partner values (partition p^32) come from partition-base-shifted
   DVE tensor_copies (engines allow differing in/out partition bases; bf16
   SBUF copies run in 4x DVE mode), replacing 128 SBUF->SBUF DMAs.
 - Weights/x load as single 3-dim-AP DMAs (e.g. [128, 8dt, cols]) instead
   of per-dt strips: ~60 DMA instructions total vs 247 (HWDGE issue and
   SP.SEQ are ~630ns/DMA serial resources).  wq/wk load their cc0 column
   slice and the first 512 cos/sin columns first so projection + RoPE for
   the first attention slot are ready ~6us in.
 - The emission interleaves attention slots (pair, j-tile) with side work:
   projections of later s-tiles (q two tiles ahead, k/v one - k/v of
   s-tile st are only needed from j-tile 4*st of attention tile st) and
   o_proj of the previous tile, paced evenly across each tile's slots and
   placed between the scores issue and the exp-blocked AV block (the PE's
   4-deep wait queue can't bypass a clogged dependent block).  Engine
   assignment of psum-draining copies alternates DVE/ACT by phase to track
   which engine has slack (DVE is the early bottleneck, ACT's exp the late
   one).
"""

import numpy as np
import ml_dtypes
from contextlib import ExitStack

import concourse.bass as bass
import concourse.bacc as bacc
import concourse.mybir as mybir
import concourse.tile as tile
from concourse.ap import AP
from concourse.bass_utils import run_bass_kernel_spmd

B, S, D = 4, 2048, 1024
H_TOT, DK = 16, 64
THETA = 10000.0
N_CORES = 8
HG = 2                  # head groups (cores per batch)
H_LOC = H_TOT // HG     # 8 heads per core
CL = H_LOC * DK         # 512 local channels
P = 128
DT = D // P             # 8 contraction tiles
CC = CL // P            # 4 c-chunks (2 heads each)
ST = S // 512           # 4 s-tiles of 512
F32 = mybir.dt.float32
F32R = mybir.dt.float32r
BF16 = mybir.dt.bfloat16
OP = mybir.AluOpType
AF = mybir.ActivationFunctionType

_CACHE: dict = {}
LAST_RESULT = None  # stashed BassKernelResults for test harness introspection


def _build_program():
    nc = bacc.Bacc(
        "TRN2", target_bir_lowering=False, debug=False,
        num_devices=N_CORES,
    )
    xT = nc.declare_dram_parameter("xT", [D, S], BF16, isOutput=False).ap()
    wq = nc.declare_dram_parameter("wq", [D, CL], BF16, isOutput=False).ap()
    wk = nc.declare_dram_parameter("wk", [D, CL], BF16, isOutput=False).ap()
    wv = nc.declare_dram_parameter("wv", [D, CL], BF16, isOutput=False).ap()
    wo = nc.declare_dram_parameter("wo", [CL, D], BF16, isOutput=False).ap()
    cosr = nc.declare_dram_parameter("cosr", [P, S], BF16, isOutput=False).ap()
    sinr = nc.declare_dram_parameter("sinr", [P, S], BF16, isOutput=False).ap()
    msk = nc.declare_dram_parameter("msk", [P, P], BF16, isOutput=False).ap()
    ones1 = nc.declare_dram_parameter("ones1", [P, 1], BF16, isOutput=False).ap()
    ident = nc.declare_dram_parameter("ident", [P, P], BF16, isOutput=False).ap()
    out = nc.declare_dram_parameter("out", [S, D], F32, isOutput=True).ap()

    with tile.TileContext(nc) as tc:
        with ExitStack() as ctx, nc.allow_low_precision(
                reason="bf16 attention operands; psum accumulation stays fp32"):
            _emit(nc, tc, ctx, xT, wq, wk, wv, wo, cosr, sinr, msk, ones1, ident, out)
    nc.finalize()
    return nc


def _emit(nc, tc, ctx, xT, wq, wk, wv, wo, cosr, sinr, msk, ones1, ident, out):
    consts = ctx.enter_context(tc.tile_pool(name="consts", bufs=1))
    kv_pool = ctx.enter_context(tc.tile_pool(name="kv", bufs=1))
    xt_pool = ctx.enter_context(tc.tile_pool(name="xt", bufs=3))
    qt_pool = ctx.enter_context(tc.tile_pool(name="qt", bufs=3))
    rope_pool = ctx.enter_context(tc.tile_pool(name="rope", bufs=2))
    pt_pool = ctx.enter_context(tc.tile_pool(name="pt", bufs=4))
    on_pool = ctx.enter_context(tc.tile_pool(name="on", bufs=3))
    rec_pool = ctx.enter_context(tc.tile_pool(name="rec", bufs=2))
    ot_pool = ctx.enter_context(tc.tile_pool(name="ot", bufs=3))
    ob_pool = ctx.enter_context(tc.tile_pool(name="ob", bufs=2))
    sc_ps = ctx.enter_context(tc.tile_pool(name="scps", bufs=2, space="PSUM"))
    av_ps = ctx.enter_context(tc.tile_pool(name="avps", bufs=1, space="PSUM"))
    ad_ps = ctx.enter_context(tc.tile_pool(name="adps", bufs=1, space="PSUM"))
    tr_ps = ctx.enter_context(tc.tile_pool(name="trps", bufs=1, space="PSUM"))
    pp_ps = ctx.enter_context(tc.tile_pool(name="pp", bufs=1, space="PSUM"))

    # ---- resident constants --------------------------------------------
    wq_sb = consts.tile([P, DT, CL], BF16, name="wq_sb")
    wk_sb = consts.tile([P, DT, CL], BF16, name="wk_sb")
    wv_sb = consts.tile([P, DT, CL], BF16, name="wv_sb")
    wo_sb = consts.tile([P, CC, D], BF16, name="wo_sb")
    cos_sb = consts.tile([P, S], BF16, name="cos_sb")
    sin_sb = consts.tile([P, S], BF16, name="sin_sb")
    msk_sb = consts.tile([P, P], BF16, name="msk_sb")
    ones_sb = consts.tile([P, 1], BF16, name="ones_sb")
    id_sb = consts.tile([P, P], BF16, name="id_sb")
    kT_t = [kv_pool.tile([P, CC, 512], BF16, name=f"kT{st}") for st in range(ST)]
    vv_t = [kv_pool.tile([P, H_LOC, DK], BF16, name=f"vv{j}")
            for j in range(S // P)]

    def w_slice_dma(dst_sb, w_ap, c0, c1):
        # dst_sb[:, dt, c0:c1] <- w[dt*128+p, c0:c1] in ONE 3-dim DMA
        src = AP(w_ap.tensor, w_ap.offset + c0, [[CL, P], [CL * P, DT], [1, c1 - c0]])
        nc.sync.dma_start(dst_sb[:, :, c0:c1], src)

    def load_consts():
        # ordered for fast compute start: first proj group is q(cc0, st0)
        w_slice_dma(wq_sb, wq, 0, P)
        w_slice_dma(wk_sb, wk, 0, P)
        for dt in range(DT):  # xt st0 per-dt so q0/k0 accumulation chases arrivals
            nc.sync.dma_start(
                xt0[:, dt, :], xT[dt * P:(dt + 1) * P, 0:512])
        # first 512 cols of cos/sin unblock st0's RoPE ~5us earlier
        nc.sync.dma_start(cos_sb[:, 0:512], cosr[:, 0:512])
        nc.sync.dma_start(sin_sb[:, 0:512], sinr[:, 0:512])
        for dt in range(DT):  # per-dt so v0's accumulation chases arrivals
            nc.sync.dma_start(
                wv_sb[:, dt, :], wv[dt * P:(dt + 1) * P, :])
        nc.sync.dma_start(cos_sb[:, 512:S], cosr[:, 512:S])
        nc.sync.dma_start(sin_sb[:, 512:S], sinr[:, 512:S])
        nc.sync.dma_start(msk_sb[:], msk)
        nc.sync.dma_start(ones_sb[:], ones1)
        nc.sync.dma_start(id_sb[:], ident)
        w_slice_dma(wq_sb, wq, P, CL)
        w_slice_dma(wk_sb, wk, P, CL)
        wo_src = AP(wo.tensor, wo.offset, [[D, P], [D * P, CC], [1, D]])
        nc.sync.dma_start(wo_sb[:], wo_src)

    from collections import deque

    def load_st(st):
        s0 = st * 512
        xt = xt_pool.tile([P, DT, 512], BF16, name="xt")
        src = AP(xT.tensor, xT.offset + s0, [[S, P], [S * P, DT], [1, 512]])
        nc.sync.dma_start(xt[:], src)
        qTs = qt_pool.tile([P, CC, 512], BF16, name="qTs")
        return xt, qTs

    def qk_group(st, cc, which, ctx_tiles):
        xt, qTs = ctx_tiles
        s0 = st * 512
        w_sb = wq_sb if which == "q" else wk_sb
        dst = qTs[:, cc, :] if which == "q" else kT_t[st][:, cc, :]
        ps = pp_ps.tile([P, 512], F32, name="pp")
        for dt in range(DT):
            nc.tensor.matmul(
                ps[:], lhsT=(w_sb[:, dt, cc * P:(cc + 1) * P]),
                rhs=(xt[:, dt, :]), start=(dt == 0), stop=(dt == DT - 1))
        qraw = rope_pool.tile([P, 512], BF16, name="qraw")
        # psum->sbuf copy on ACT for early s-tiles (ACT idles while DVE is
        # the early bottleneck); DVE for late ones (ACT saturates on exp)
        if st <= 1:
            nc.scalar.copy(qraw[:], ps[:])
        else:
            nc.vector.tensor_copy(qraw[:], ps[:])
        # partner view (partition p ^ 32) via partition-base-shifted DVE
        # copies (bf16 SBUF copies run in 4x DVE mode)
        qsh = rope_pool.tile([P, 512], BF16, name="qsh")
        for blk in range(4):
            p0 = blk * 32
            q0 = p0 ^ 32
            nc.vector.tensor_copy(qsh[p0:p0 + 32, :], qraw[q0:q0 + 32, :])
        tcos = rope_pool.tile([P, 512], BF16, name="tcos")
        nc.gpsimd.tensor_tensor(tcos[:], qraw[:], cos_sb[:, s0:s0 + 512], OP.mult)
        nc.vector.tensor_tensor(dst, qsh[:], sin_sb[:, s0:s0 + 512], OP.mult)
        nc.vector.tensor_tensor(dst, dst, tcos[:], OP.add)

    def v_group(st, sc, ctx_tiles):
        xt = ctx_tiles[0]
        gsc = st * 4 + sc
        ps = pp_ps.tile([P, 512], F32, name="pp")
        for dt in range(DT):
            nc.tensor.matmul(
                ps[:], lhsT=(xt[:, dt, sc * P:(sc + 1) * P]),
                rhs=(wv_sb[:, dt, :]), start=(dt == 0), stop=(dt == DT - 1))
        if st <= 1:
            nc.scalar.copy(
                vv_t[gsc][:], ps.rearrange("p (h c) -> p h c", c=DK))
        else:
            nc.vector.tensor_copy(
                vv_t[gsc][:], ps.rearrange("p (h c) -> p h c", c=DK))

    def oproj_group(ti, oT_blks, et, sc4, use_sc_pool=False, tail=False):
        s0 = ti * 512
        if use_sc_pool:
            ps = sc_ps.tile([P, 512], F32, name="sc")
        else:
            ps = pp_ps.tile([P, 512], F32, name="pp")
        for cc in range(CC):
            nc.tensor.matmul(
                ps[:], lhsT=(oT_blks[cc][sc4][:]),
                rhs=(wo_sb[:, cc, et * 512:(et + 1) * 512]),
                start=(cc == 0), stop=(cc == CC - 1))
        osb = ob_pool.tile([P, 512], F32, name="osb")
        rows = slice(s0 + sc4 * P, s0 + (sc4 + 1) * P)
        if tail:
            # epilogue: ACT and HWDGE are idle - split the copy and the
            # out-DMA in halves so the final chain pipelines
            nc.vector.tensor_copy(osb[:, 0:256], ps[:, 0:256])
            nc.sync.dma_start(
                out[rows, et * 512:et * 512 + 256], osb[:, 0:256])
            nc.scalar.copy(osb[:, 256:512], ps[:, 256:512])
            nc.sync.dma_start(
                out[rows, et * 512 + 256:(et + 1) * 512], osb[:, 256:512])
        else:
            # psum->sbuf staging on DVE (ACT stays exp-only; GPSIMD cannot
            # read PSUM)
            nc.vector.tensor_copy(osb[:], ps[:])
            nc.sync.dma_start(
                out[rows, et * 512:(et + 1) * 512], osb[:])

    # prologue: minimal inline projections (q-cc0, k-cc0, v0); the rest of
    # st0's projections go through the global side queue so attention(ti=0)
    # starts ~7us in
    xt0 = xt_pool.tile([P, DT, 512], BF16, name="xt")
    qTs0 = qt_pool.tile([P, CC, 512], BF16, name="qTs")
    ctx0 = (xt0, qTs0)
    load_consts()
    qk_group(0, 0, "q", ctx0)
    qk_group(0, 0, "k", ctx0)
    v_group(0, 0, ctx0)
    st_tiles = {0: ctx0}

    # ------------------------------------------------------------------
    # Global side-work queue.  k/v projections of s-tile st are legal to
    # run DURING attention(ti=st) up to jt=4*st; q projections of st must
    # finish before ti=st starts.  o_proj(ti) runs during ti+1.
    # ------------------------------------------------------------------
    side = deque()
    deferred_oproj = None

    def seed_side_for_ti(ti):
        # called at ti start: queue o_proj(ti-1), then k/v(st=ti) early-need
        # groups were queued at ti-1... layout below.
        pass

    prev_oproj = None
    for ti in range(ST):
        qTs = st_tiles[ti][1]
        # deferred k/v of THIS s-tile first (needed from jt=4*ti onward),
        # then o_proj of the previous tile, then next tile's q projections.
        if ti >= 1:
            ctxn = st_tiles[ti]
            side.appendleft((qk_group, ti, 0, "k", ctxn))
            pos = 1
            for sc in range(4):
                side.insert(pos, (v_group, ti, sc, ctxn))
                pos += 1
            for cc in range(1, CC):
                side.insert(pos, (qk_group, ti, cc, "k", ctxn))
                pos += 1
        if deferred_oproj is not None:
            pti, poT = deferred_oproj
            for sc4 in range(4):
                side.append((oproj_group, pti, poT, 1, sc4))
            deferred_oproj = None
        if prev_oproj is not None:
            pti, poT = prev_oproj
            for et in range(2):
                for sc4 in range(4):
                    if pti == 1 and et == 1:
                        continue  # deferred into ti3
                    side.append((oproj_group, pti, poT, et, sc4))
            if pti == 1:
                deferred_oproj = (pti, poT)
            prev_oproj = None
        if ti + 1 < ST and ti + 1 not in st_tiles:
            nxt = load_st(ti + 1)
            st_tiles[ti + 1] = nxt
            if ti + 1 == 1:
                # remaining st0 groups first (attention ti0 needs them)
                side.extend([(v_group, 0, sc, ctx0) for sc in (1, 2, 3)])
                side.extend([(qk_group, 0, cc, w, ctx0)
                             for cc in (1, 2, 3) for w in ("q", "k")])
            if ti == 0:
                # q projections of st1 (must complete before ti1)
                side.extend([(qk_group, ti + 1, cc, "q", nxt)
                             for cc in range(CC)])
        if ti + 2 < ST and ti + 2 not in st_tiles:
            nxt2 = load_st(ti + 2)
            st_tiles[ti + 2] = nxt2
            # q projections of st+2 run in THIS tile's tail (they only need
            # xt(st+2), an input) so ti+1 keeps its slots for k/v + o_proj
            side.extend([(qk_group, ti + 2, cc, "q", nxt2)
                         for cc in range(CC)])

        oT_blks = [[ot_pool.tile([P, P], BF16, name=f"oTb_{pr}_{b}")
                    for b in range(4)] for pr in range(CC)]
        ad = ad_ps.tile([P, 32], F32, name="ad")
        njt = 4 * (ti + 1)
        slots = [(pair, jt) for pair in range(CC) for jt in range(njt)]
        state = {"ad_started": False, "av": None, "av_started": False,
                 "rec": None, "pt": {}, "sc": {}}

        def phase1(slot):
            pair, jt = slot
            r = max(0, (jt - 4 * ti) * P)
            kslc = kT_t[jt // 4][:, pair, (jt % 4) * P:(jt % 4) * P + P]
            scb = sc_ps.tile([P, 1024], F32, name="sc")
            nc.tensor.matmul(
                scb[:, r:512], lhsT=(kslc[0:DK]),
                rhs=(qTs[0:DK, pair, r:512]), start=True, stop=True)
            nc.tensor.matmul(
                scb[:, 512 + r:1024], lhsT=(kslc[DK:P]),
                rhs=(qTs[DK:P, pair, r:512]), start=True, stop=True,
                skip_group_check=True)
            pt = pt_pool.tile([P, 1024], BF16, name="pt")
            if r >= 256:
                # deep-diagonal slots: two exps beat paying for the dead gap
                nc.scalar.activation(
                    pt[:, r:512], scb[:, r:512], AF.Exp, scale=0.125)
                nc.scalar.activation(
                    pt[:, 512 + r:1024], scb[:, 512 + r:1024], AF.Exp,
                    scale=0.125)
            else:
                # one bank-spanning exp covering both heads' ranges; the gap
                # cols [512:512+r) hold stale psum whose exp is never read
                nc.scalar.activation(
                    pt[:, r:1024], scb[:, r:1024], AF.Exp, scale=0.125)
            state["sc"][slot] = scb
            if jt >= 4 * ti:
                nc.vector.tensor_tensor(
                    pt[:, r:r + P], pt[:, r:r + P], msk_sb[:], OP.mult)
                nc.vector.tensor_tensor(
                    pt[:, 512 + r:512 + r + P], pt[:, 512 + r:512 + r + P],
                    msk_sb[:], OP.mult)
            state["pt"][slot] = pt

        def phase2(slot):
            pair, jt = slot
            pt = state["pt"].pop(slot)
            state["sc"].pop(slot)
            if jt == 0:
                state["av"] = av_ps.tile([P, 512], F32, name="av")
                state["av_started"] = False
                state["rec"] = rec_pool.tile([P, 8], F32, name="rec")
            av, rec = state["av"], state["rec"]
            ib0 = max(0, jt - 4 * ti)
            # h0 AV matmuls first (they only wait on the h0 exp), then h1
            for hh in range(2):
                for ib in range(ib0, 4):
                    last = (jt == 4 * ti + ib)
                    h = 2 * pair + hh
                    lhs = (pt[:, ib * P:ib * P + P] if hh == 0
                           else pt[:, 512 + ib * P:512 + ib * P + P])
                    nc.tensor.matmul(
                        av[:, ib * P + hh * DK:ib * P + hh * DK + DK],
                        lhsT=(lhs), rhs=(vv_t[jt][:, h, :]),
                        start=(not state["av_started"]), stop=last,
                        skip_group_check=True)
                    state["av_started"] = True
                    dc = pair * 8 + ib * 2 + hh
                    nc.tensor.matmul(
                        ad[:, dc:dc + 1], lhsT=(lhs), rhs=(ones_sb[:]),
                        start=(not state["ad_started"]), stop=last,
                        skip_group_check=True)
                    state["ad_started"] = True
            ibd = jt - 4 * ti
            if 0 <= ibd <= 3:
                ib = ibd
                # drain (pair, ib): per-partition normalize, PE transpose,
                # copy out to the SBUF lhsT tile for o_proj
                c2 = pair * 8 + ib * 2
                nc.vector.reciprocal(
                    rec[:, ib * 2:ib * 2 + 2], ad[:, c2:c2 + 2])
                o_n = on_pool.tile([P, P], BF16, name="o_n")
                nc.vector.tensor_scalar(
                    o_n[:, 0:DK],
                    av[:, ib * P:ib * P + DK],
                    rec[:, ib * 2:ib * 2 + 1], None, OP.mult)
                nc.vector.tensor_scalar(
                    o_n[:, DK:2 * DK],
                    av[:, ib * P + DK:ib * P + 2 * DK],
                    rec[:, ib * 2 + 1:ib * 2 + 2], None, OP.mult)
                trp = tr_ps.tile([P, P], BF16, name="tr")
                nc.tensor.matmul(
                    trp[:], lhsT=(o_n[:]), rhs=(id_sb[:]),
                    is_transpose=True, start=True, stop=True,
                    skip_group_check=True)
                nc.vector.tensor_copy(oT_blks[pair][ib][:], trp[:])

        # depth-2 software pipeline: sc/exp of slot i+2 issue before the AV
        # block of slot i, so exp latency hides behind PE work
        # ti3's own o_proj for i-block column sc4 only needs pair3's ib=sc4
        # drain (at slot 60+sc4); pull the early columns into the last slots
        tail_oproj = deque()
        if ti == ST - 1:
            for sc4 in range(4):
                for et in range(2):
                    tail_oproj.append((oproj_group, ti, oT_blks, et, sc4))

        nslots = len(slots)
        side_n = len(side)
        popped = 0
        for i in range(nslots):
            phase1(slots[i])
            # side work lands between the scores/exp issue and the
            # exp-blocked AV block (the PE's 4-deep wait queue means work
            # emitted after phase2 can't bypass the stalled AV matmuls),
            # paced evenly across the tile's slots
            want = -(-side_n * (i + 1) // nslots)  # ceil
            while popped < want and side:
                g = side.popleft()
                g[0](*g[1:])
                popped += 1
            phase2(slots[i])
            if ti == ST - 1 and i >= 61 and tail_oproj:
                for _ in range(1 if i == 61 else 2):
                    if tail_oproj:
                        g = tail_oproj.popleft()
                        g[0](*g[1:])
        while side:
            g = side.popleft()
            g[0](*g[1:])
        prev_oproj = (ti, oT_blks)

    # final o_proj (whatever ti3 didn't absorb): alternate between the pp
    # bank and the now-idle scores banks so the tail pipelines
    use_sc = False
    while tail_oproj:
        g = tail_oproj.popleft()
        g[0](*g[1:], use_sc_pool=use_sc, tail=True)
        use_sc = not use_sc


def _host_prep(x, Wq, Wk, Wv, Wo, token_positions):
    """Build the 8 per-core input maps (sharding + layout prep only)."""
    x = np.asarray(x, dtype=np.float32)
    Wq = np.asarray(Wq, dtype=np.float32)
    Wk = np.asarray(Wk, dtype=np.float32)
    Wv = np.asarray(Wv, dtype=np.float32)
    Wo = np.asarray(Wo, dtype=np.float32)
    pos = np.asarray(token_positions)

    half = DK // 2
    inv_freq = THETA ** (-np.arange(half, dtype=np.float64) * 2.0 / DK)
    ang = pos.astype(np.float64)[None, :] * inv_freq[:, None]      # [32, S]
    cos32 = np.cos(ang)
    sin32 = np.sin(ang)
    cosr = np.empty((P, S), dtype=np.float32)
    sinr = np.empty((P, S), dtype=np.float32)
    for p in range(P):
        ip = p % DK
        i = ip % half
        cosr[p] = cos32[i]
        sinr[p] = (-sin32[i]) if ip < half else sin32[i]
    cosr = cosr.astype(ml_dtypes.bfloat16)
    sinr = sinr.astype(ml_dtypes.bfloat16)

    # de-interleave permutation within each head: [evens | odds]
    perm = np.concatenate([np.arange(0, DK, 2), np.arange(1, DK, 2)])

    msk = np.triu(np.ones((P, P), dtype=np.float32)).astype(ml_dtypes.bfloat16)
    ones1 = np.ones((P, 1), dtype=np.float32).astype(ml_dtypes.bfloat16)
    ident = np.eye(P, dtype=np.float32).astype(ml_dtypes.bfloat16)

    WqT = Wq.T  # [d_in, e_out]
    WkT = Wk.T
    WvT = Wv.T
    WoT = Wo.T  # [e_in, d_out]

    in_maps = []
    for core in range(N_CORES):
        b, g = core // HG, core % HG
        cols = np.concatenate(
            [g * CL + h * DK + perm for h in range(H_LOC)])
        plain = slice(g * CL, (g + 1) * CL)
        in_maps.append({
            "xT": np.ascontiguousarray(x[b].T).astype(ml_dtypes.bfloat16),
            "wq": np.ascontiguousarray(WqT[:, cols]).astype(ml_dtypes.bfloat16),
            "wk": np.ascontiguousarray(WkT[:, cols]).astype(ml_dtypes.bfloat16),
            "wv": np.ascontiguousarray(WvT[:, plain]).astype(ml_dtypes.bfloat16),
            "wo": np.ascontiguousarray(WoT[plain, :]).astype(ml_dtypes.bfloat16),
            "cosr": cosr,
            "sinr": sinr,
            "msk": msk,
            "ones1": ones1,
            "ident": ident,
        })
    return in_maps


def kernel(x, Wq, Wk, Wv, Wo, token_positions, _trace=False):
    global LAST_RESULT
    if "nc" not in _CACHE:
        _CACHE["nc"] = _build_program()
    nc = _CACHE["nc"]

    in_maps = _host_prep(x, Wq, Wk, Wv, Wo, token_positions)
    res = run_bass_kernel_spmd(nc, in_maps, core_ids=list(range(N_CORES)),
                               trace=_trace)
    LAST_RESULT = res
    outs = [r["out"] for r in res.results]
    final = np.empty((B, S, D), dtype=np.float32)
    for b in range(B):
        final[b] = outs[b * HG]
        for g in range(1, HG):
            final[b] += outs[b * HG + g]
    return final


# revision 50
# speedup vs baseline: 1.0115x; 1.0078x over previous
"""Trainium2 Bass kernel: causal multi-head self-attention with RoPE.

Problem: B=4, S=2048, D=1024, H=16 heads, dk=64, fp32.
Sharding: 8 cores = (batch b in 0..3) x (head-group g in 0..1, 8 heads each).
Each core computes a partial o_proj output [S, D] for its (b, g); the host
sums the two head-group partials per batch and stacks batches.

Design (vs the 383us fp32r baseline; this version sims at ~261us):
 - Everything downstream of HBM runs on bf16 operands (x, all weights, q/k
   post-RoPE, exp-scores, v, o) with fp32 psum accumulation: bf16 matmuls
   run at 1 cycle/row at ANY moving size, unlike fp32r which pays 4x under
   moving=256, enabling the flipped AV orientation below.  End-to-end L2
   err ~6e-3 (gate 2e-2).
 - AV matmuls are flipped to out[i-block 128 part, dk moving]: lhsT =
   exp-scores tile pt[j, i-block], rhs = v[j, dk].  Cost is charged per
   moving row, so this halves AV PE time vs the [dk+1 part, i moving]
   orientation (full 128-partition output occupancy).  A parallel 1-col
   matmul against a ones vector accumulates the softmax denominators into a
   separate psum bank, making them per-PARTITION scalars; normalize is then
   reciprocal [128,2] + tensor_scalar multiply, and oT for o_proj comes
   from [128,128] PE transposes (53ns each in bf16) into per-(pair,i-block)
   contiguous SBUF lhsT tiles.
 - Scores for both heads of a pair land in one [128,1024] 2-bank psum tile
   and a single bank-spanning exp converts them (185ns/instr access latency
   amortized; deep-diagonal slots split in two to skip the dead gap).
 - PSUM banks (8): 2x scores [128,1024] + 1x AV [128,512] + 1x denom
   [128,32] + 1x transpose + 1x proj/oproj [128,512] (+padding).  Multiple
   accumulation groups share the AV/denom banks with start=True only on the
   bank's first matmul (the backend's pending-zero region is bank-wide;
   later groups' first touch overwrites, then accumulates).
 - RoPE partner values (partition p^32) come from partition-base-shifted
   DVE tensor_copies (engines allow differing in/out partition bases; bf16
   SBUF copies run in 4x DVE mode), replacing 128 SBUF->SBUF DMAs.
 - Weights/x load as single 3-dim-AP DMAs (e.g. [128, 8dt, cols]) instead
   of per-dt strips: ~60 DMA instructions total vs 247 (HWDGE issue and
   SP.SEQ are ~630ns/DMA serial resources).  wq/wk load their cc0 column
   slice and the first 512 cos/sin columns first so projection + RoPE for
   the first attention slot are ready ~6us in.
 - The emission interleaves attention slots (pair, j-tile) with side work:
   projections of later s-tiles (q two tiles ahead, k/v one - k/v of
   s-tile st are only needed from j-tile 4*st of attention tile st) and
   o_proj of the previous tile, paced evenly across each tile's slots and
   placed between the scores issue and the exp-blocked AV block (the PE's
   4-deep wait queue can't bypass a clogged dependent block).  Engine
   assignment of psum-draining copies alternates DVE/ACT by phase to track
   which engine has slack (DVE is the early bottleneck, ACT's exp the late
   one).
"""

import numpy as np
import ml_dtypes
from contextlib import ExitStack

import concourse.bass as bass
import concourse.bacc as bacc
import concourse.mybir as mybir
import concourse.tile as tile
from concourse.ap import AP
from concourse.bass_utils import run_bass_kernel_spmd

B, S, D = 4, 2048, 1024
H_TOT, DK = 16, 64
THETA = 10000.0
N_CORES = 8
HG = 2                  # head groups (cores per batch)
H_LOC = H_TOT // HG     # 8 heads per core
CL = H_LOC * DK         # 512 local channels
P = 128
DT = D // P             # 8 contraction tiles
CC = CL // P            # 4 c-chunks (2 heads each)
ST = S // 512           # 4 s-tiles of 512
F32 = mybir.dt.float32
F32R = mybir.dt.float32r
BF16 = mybir.dt.bfloat16
OP = mybir.AluOpType
AF = mybir.ActivationFunctionType

_CACHE: dict = {}
LAST_RESULT = None  # stashed BassKernelResults for test harness introspection


def _build_program():
    nc = bacc.Bacc(
        "TRN2", target_bir_lowering=False, debug=False,
        num_devices=N_CORES,
    )
    xT = nc.declare_dram_parameter("xT", [D, S], BF16, isOutput=False).ap()
    wq = nc.declare_dram_parameter("wq", [D, CL], BF16, isOutput=False).ap()
    wk = nc.declare_dram_parameter("wk", [D, CL], BF16, isOutput=False).ap()
    wv = nc.declare_dram_parameter("wv", [D, CL], BF16, isOutput=False).ap()
    wo = nc.declare_dram_parameter("wo", [CL, D], BF16, isOutput=False).ap()
    cosr = nc.declare_dram_parameter("cosr", [P, S], BF16, isOutput=False).ap()
    sinr = nc.declare_dram_parameter("sinr", [P, S], BF16, isOutput=False).ap()
    msk = nc.declare_dram_parameter("msk", [P, P], BF16, isOutput=False).ap()
    ones1 = nc.declare_dram_parameter("ones1", [P, 1], BF16, isOutput=False).ap()
    ident = nc.declare_dram_parameter("ident", [P, P], BF16, isOutput=False).ap()
    out = nc.declare_dram_parameter("out", [S, D], F32, isOutput=True).ap()

    with tile.TileContext(nc) as tc:
        with ExitStack() as ctx, nc.allow_low_precision(
                reason="bf16 attention operands; psum accumulation stays fp32"):
            _emit(nc, tc, ctx, xT, wq, wk, wv, wo, cosr, sinr, msk, ones1, ident, out)
    nc.finalize()
    return nc


def _emit(nc, tc, ctx, xT, wq, wk, wv, wo, cosr, sinr, msk, ones1, ident, out):
    consts = ctx.enter_context(tc.tile_pool(name="consts", bufs=1))
    kv_pool = ctx.enter_context(tc.tile_pool(name="kv", bufs=1))
    xt_pool = ctx.enter_context(tc.tile_pool(name="xt", bufs=3))
    qt_pool = ctx.enter_context(tc.tile_pool(name="qt", bufs=3))
    rope_pool = ctx.enter_context(tc.tile_pool(name="rope", bufs=3))
    pt_pool = ctx.enter_context(tc.tile_pool(name="pt", bufs=4))
    on_pool = ctx.enter_context(tc.tile_pool(name="on", bufs=4))
    rec_pool = ctx.enter_context(tc.tile_pool(name="rec", bufs=3))
    ot_pool = ctx.enter_context(tc.tile_pool(name="ot", bufs=3))
    ob_pool = ctx.enter_context(tc.tile_pool(name="ob", bufs=3))
    sc_ps = ctx.enter_context(tc.tile_pool(name="scps", bufs=2, space="PSUM"))
    av_ps = ctx.enter_context(tc.tile_pool(name="avps", bufs=1, space="PSUM"))
    ad_ps = ctx.enter_context(tc.tile_pool(name="adps", bufs=1, space="PSUM"))
    tr_ps = ctx.enter_context(tc.tile_pool(name="trps", bufs=1, space="PSUM"))
    pp_ps = ctx.enter_context(tc.tile_pool(name="pp", bufs=1, space="PSUM"))

    # ---- resident constants --------------------------------------------
    wq_sb = consts.tile([P, DT, CL], BF16, name="wq_sb")
    wk_sb = consts.tile([P, DT, CL], BF16, name="wk_sb")
    wv_sb = consts.tile([P, DT, CL], BF16, name="wv_sb")
    wo_sb = consts.tile([P, CC, D], BF16, name="wo_sb")
    cos_sb = consts.tile([P, S], BF16, name="cos_sb")
    sin_sb = consts.tile([P, S], BF16, name="sin_sb")
    msk_sb = consts.tile([P, P], BF16, name="msk_sb")
    ones_sb = consts.tile([P, 1], BF16, name="ones_sb")
    id_sb = consts.tile([P, P], BF16, name="id_sb")
    kT_t = [kv_pool.tile([P, CC, 512], BF16, name=f"kT{st}") for st in range(ST)]
    vv_t = [kv_pool.tile([P, H_LOC, DK], BF16, name=f"vv{j}")
            for j in range(S // P)]

    def w_slice_dma(dst_sb, w_ap, c0, c1):
        # dst_sb[:, dt, c0:c1] <- w[dt*128+p, c0:c1] in ONE 3-dim DMA
        src = AP(w_ap.tensor, w_ap.offset + c0, [[CL, P], [CL * P, DT], [1, c1 - c0]])
        nc.sync.dma_start(dst_sb[:, :, c0:c1], src)

    def load_consts():
        # ordered for fast compute start: first proj group is q(cc0, st0)
        w_slice_dma(wq_sb, wq, 0, P)
        w_slice_dma(wk_sb, wk, 0, P)
        for dt in range(DT):  # xt st0 per-dt so q0/k0 accumulation chases arrivals
            nc.sync.dma_start(
                xt0[:, dt, :], xT[dt * P:(dt + 1) * P, 0:512])
        # first 512 cols of cos/sin unblock st0's RoPE ~5us earlier
        nc.sync.dma_start(cos_sb[:, 0:512], cosr[:, 0:512])
        nc.sync.dma_start(sin_sb[:, 0:512], sinr[:, 0:512])
        for dt in range(DT):  # per-dt so v0's accumulation chases arrivals
            nc.sync.dma_start(
                wv_sb[:, dt, :], wv[dt * P:(dt + 1) * P, :])
        nc.sync.dma_start(cos_sb[:, 512:S], cosr[:, 512:S])
        nc.sync.dma_start(sin_sb[:, 512:S], sinr[:, 512:S])
        nc.sync.dma_start(msk_sb[:], msk)
        nc.sync.dma_start(ones_sb[:], ones1)
        nc.sync.dma_start(id_sb[:], ident)
        w_slice_dma(wq_sb, wq, P, CL)
        w_slice_dma(wk_sb, wk, P, CL)
        wo_src = AP(wo.tensor, wo.offset, [[D, P], [D * P, CC], [1, D]])
        nc.sync.dma_start(wo_sb[:], wo_src)

    from collections import deque

    def load_st(st):
        s0 = st * 512
        xt = xt_pool.tile([P, DT, 512], BF16, name="xt")
        src = AP(xT.tensor, xT.offset + s0, [[S, P], [S * P, DT], [1, 512]])
        nc.sync.dma_start(xt[:], src)
        qTs = qt_pool.tile([P, CC, 512], BF16, name="qTs")
        return xt, qTs

    def qk_group(st, cc, which, ctx_tiles):
        xt, qTs = ctx_tiles
        s0 = st * 512
        w_sb = wq_sb if which == "q" else wk_sb
        dst = qTs[:, cc, :] if which == "q" else kT_t[st][:, cc, :]
        ps = pp_ps.tile([P, 512], F32, name="pp")
        for dt in range(DT):
            nc.tensor.matmul(
                ps[:], lhsT=(w_sb[:, dt, cc * P:(cc + 1) * P]),
                rhs=(xt[:, dt, :]), start=(dt == 0), stop=(dt == DT - 1))
        qraw = rope_pool.tile([P, 512], BF16, name="qraw")
        # psum->sbuf copy on ACT for early s-tiles (ACT idles while DVE is
        # the early bottleneck); DVE for late ones (ACT saturates on exp)
        if st <= 1:
            nc.scalar.copy(qraw[:], ps[:])
        else:
            nc.vector.tensor_copy(qraw[:], ps[:])
        # partner view (partition p ^ 32) via partition-base-shifted DVE
        # copies (bf16 SBUF copies run in 4x DVE mode)
        qsh = rope_pool.tile([P, 512], BF16, name="qsh")
        for blk in range(4):
            p0 = blk * 32
            q0 = p0 ^ 32
            nc.vector.tensor_copy(qsh[p0:p0 + 32, :], qraw[q0:q0 + 32, :])
        tcos = rope_pool.tile([P, 512], BF16, name="tcos")
        nc.gpsimd.tensor_tensor(tcos[:], qraw[:], cos_sb[:, s0:s0 + 512], OP.mult)
        nc.vector.tensor_tensor(dst, qsh[:], sin_sb[:, s0:s0 + 512], OP.mult)
        nc.vector.tensor_tensor(dst, dst, tcos[:], OP.add)

    def v_group(st, sc, ctx_tiles):
        xt = ctx_tiles[0]
        gsc = st * 4 + sc
        ps = pp_ps.tile([P, 512], F32, name="pp")
        for dt in range(DT):
            nc.tensor.matmul(
                ps[:], lhsT=(xt[:, dt, sc * P:(sc + 1) * P]),
                rhs=(wv_sb[:, dt, :]), start=(dt == 0), stop=(dt == DT - 1))
        if st <= 1:
            nc.scalar.copy(
                vv_t[gsc][:], ps.rearrange("p (h c) -> p h c", c=DK))
        else:
            nc.vector.tensor_copy(
                vv_t[gsc][:], ps.rearrange("p (h c) -> p h c", c=DK))

    def oproj_group(ti, oT_blks, et, sc4, use_sc_pool=False, tail=False):
        s0 = ti * 512
        if use_sc_pool:
            ps = sc_ps.tile([P, 512], F32, name="sc")
        else:
            ps = pp_ps.tile([P, 512], F32, name="pp")
        for cc in range(CC):
            nc.tensor.matmul(
                ps[:], lhsT=(oT_blks[cc][sc4][:]),
                rhs=(wo_sb[:, cc, et * 512:(et + 1) * 512]),
                start=(cc == 0), stop=(cc == CC - 1))
        osb = ob_pool.tile([P, 512], F32, name="osb")
        rows = slice(s0 + sc4 * P, s0 + (sc4 + 1) * P)
        if tail:
            # epilogue: ACT and HWDGE are idle - split the copy and the
            # out-DMA in halves so the final chain pipelines
            nc.vector.tensor_copy(osb[:, 0:256], ps[:, 0:256])
            nc.sync.dma_start(
                out[rows, et * 512:et * 512 + 256], osb[:, 0:256])
            nc.scalar.copy(osb[:, 256:512], ps[:, 256:512])
            nc.sync.dma_start(
                out[rows, et * 512 + 256:(et + 1) * 512], osb[:, 256:512])
        else:
            # psum->sbuf staging on DVE (ACT stays exp-only; GPSIMD cannot
            # read PSUM)
            nc.vector.tensor_copy(osb[:], ps[:])
            nc.sync.dma_start(
                out[rows, et * 512:(et + 1) * 512], osb[:])

    # prologue: minimal inline projections (q-cc0, k-cc0, v0); the rest of
    # st0's projections go through the global side queue so attention(ti=0)
    # starts ~7us in
    xt0 = xt_pool.tile([P, DT, 512], BF16, name="xt")
    qTs0 = qt_pool.tile([P, CC, 512], BF16, name="qTs")
    ctx0 = (xt0, qTs0)
    load_consts()
    qk_group(0, 0, "q", ctx0)
    qk_group(0, 0, "k", ctx0)
    v_group(0, 0, ctx0)
    st_tiles = {0: ctx0}

    # ------------------------------------------------------------------
    # Global side-work queue.  k/v projections of s-tile st are legal to
    # run DURING attention(ti=st) up to jt=4*st; q projections of st must
    # finish before ti=st starts.  o_proj(ti) runs during ti+1.
    # ------------------------------------------------------------------
    side = deque()
    deferred_oproj = None

    def seed_side_for_ti(ti):
        # called at ti start: queue o_proj(ti-1), then k/v(st=ti) early-need
        # groups were queued at ti-1... layout below.
        pass

    prev_oproj = None
    for ti in range(ST):
        qTs = st_tiles[ti][1]
        # deferred k/v of THIS s-tile first (needed from jt=4*ti onward),
        # then o_proj of the previous tile, then next tile's q projections.
        if ti >= 1:
            ctxn = st_tiles[ti]
            side.appendleft((qk_group, ti, 0, "k", ctxn))
            pos = 1
            for sc in range(4):
                side.insert(pos, (v_group, ti, sc, ctxn))
                pos += 1
            for cc in range(1, CC):
                side.insert(pos, (qk_group, ti, cc, "k", ctxn))
                pos += 1
        if deferred_oproj is not None:
            pti, poT = deferred_oproj
            for sc4 in range(4):
                side.append((oproj_group, pti, poT, 1, sc4))
            deferred_oproj = None
        if prev_oproj is not None:
            pti, poT = prev_oproj
            for et in range(2):
                for sc4 in range(4):
                    if pti == 1 and et == 1:
                        continue  # deferred into ti3
                    side.append((oproj_group, pti, poT, et, sc4))
            if pti == 1:
                deferred_oproj = (pti, poT)
            prev_oproj = None
        if ti + 1 < ST and ti + 1 not in st_tiles:
            nxt = load_st(ti + 1)
            st_tiles[ti + 1] = nxt
            if ti + 1 == 1:
                # remaining st0 groups first (attention ti0 needs them)
                side.extend([(v_group, 0, sc, ctx0) for sc in (1, 2, 3)])
                side.extend([(qk_group, 0, cc, w, ctx0)
                             for cc in (1, 2, 3) for w in ("q", "k")])
            if ti == 0:
                # q projections of st1 (must complete before ti1)
                side.extend([(qk_group, ti + 1, cc, "q", nxt)
                             for cc in range(CC)])
        if ti + 2 < ST and ti + 2 not in st_tiles:
            nxt2 = load_st(ti + 2)
            st_tiles[ti + 2] = nxt2
            # q projections of st+2 run in THIS tile's tail (they only need
            # xt(st+2), an input) so ti+1 keeps its slots for k/v + o_proj
            side.extend([(qk_group, ti + 2, cc, "q", nxt2)
                         for cc in range(CC)])

        oT_blks = [[ot_pool.tile([P, P], BF16, name=f"oTb_{pr}_{b}")
                    for b in range(4)] for pr in range(CC)]
        ad = ad_ps.tile([P, 32], F32, name="ad")
        njt = 4 * (ti + 1)
        slots = [(pair, jt) for pair in range(CC) for jt in range(njt)]
        state = {"ad_started": False, "av": None, "av_started": False,
                 "rec": None, "pt": {}, "sc": {}}

        def phase1(slot):
            pair, jt = slot
            r = max(0, (jt - 4 * ti) * P)
            kslc = kT_t[jt // 4][:, pair, (jt % 4) * P:(jt % 4) * P + P]
            scb = sc_ps.tile([P, 1024], F32, name="sc")
            nc.tensor.matmul(
                scb[:, r:512], lhsT=(kslc[0:DK]),
                rhs=(qTs[0:DK, pair, r:512]), start=True, stop=True)
            nc.tensor.matmul(
                scb[:, 512 + r:1024], lhsT=(kslc[DK:P]),
                rhs=(qTs[DK:P, pair, r:512]), start=True, stop=True,
                skip_group_check=True)
            pt = pt_pool.tile([P, 1024], BF16, name="pt")
            if r >= 256:
                # deep-diagonal slots: two exps beat paying for the dead gap
                nc.scalar.activation(
                    pt[:, r:512], scb[:, r:512], AF.Exp, scale=0.125)
                nc.scalar.activation(
                    pt[:, 512 + r:1024], scb[:, 512 + r:1024], AF.Exp,
                    scale=0.125)
            else:
                # one bank-spanning exp covering both heads' ranges; the gap
                # cols [512:512+r) hold stale psum whose exp is never read
                nc.scalar.activation(
                    pt[:, r:1024], scb[:, r:1024], AF.Exp, scale=0.125)
            state["sc"][slot] = scb
            if jt >= 4 * ti:
                nc.vector.tensor_tensor(
                    pt[:, r:r + P], pt[:, r:r + P], msk_sb[:], OP.mult)
                nc.vector.tensor_tensor(
                    pt[:, 512 + r:512 + r + P], pt[:, 512 + r:512 + r + P],
                    msk_sb[:], OP.mult)
            state["pt"][slot] = pt

        def phase2(slot):
            pair, jt = slot
            pt = state["pt"].pop(slot)
            state["sc"].pop(slot)
            if jt == 0:
                state["av"] = av_ps.tile([P, 512], F32, name="av")
                state["av_started"] = False
                state["rec"] = rec_pool.tile([P, 8], F32, name="rec")
            av, rec = state["av"], state["rec"]
            ib0 = max(0, jt - 4 * ti)
            # h0 AV matmuls first (they only wait on the h0 exp), then h1
            for hh in range(2):
                for ib in range(ib0, 4):
                    last = (jt == 4 * ti + ib)
                    h = 2 * pair + hh
                    lhs = (pt[:, ib * P:ib * P + P] if hh == 0
                           else pt[:, 512 + ib * P:512 + ib * P + P])
                    nc.tensor.matmul(
                        av[:, ib * P + hh * DK:ib * P + hh * DK + DK],
                        lhsT=(lhs), rhs=(vv_t[jt][:, h, :]),
                        start=(not state["av_started"]), stop=last,
                        skip_group_check=True)
                    state["av_started"] = True
                    dc = pair * 8 + ib * 2 + hh
                    nc.tensor.matmul(
                        ad[:, dc:dc + 1], lhsT=(lhs), rhs=(ones_sb[:]),
                        start=(not state["ad_started"]), stop=last,
                        skip_group_check=True)
                    state["ad_started"] = True
            ibd = jt - 4 * ti
            if 0 <= ibd <= 3:
                ib = ibd
                # drain (pair, ib): per-partition normalize, PE transpose,
                # copy out to the SBUF lhsT tile for o_proj
                c2 = pair * 8 + ib * 2
                nc.vector.reciprocal(
                    rec[:, ib * 2:ib * 2 + 2], ad[:, c2:c2 + 2])
                o_n = on_pool.tile([P, P], BF16, name="o_n")
                nc.vector.tensor_scalar(
                    o_n[:, 0:DK],
                    av[:, ib * P:ib * P + DK],
                    rec[:, ib * 2:ib * 2 + 1], None, OP.mult)
                nc.vector.tensor_scalar(
                    o_n[:, DK:2 * DK],
                    av[:, ib * P + DK:ib * P + 2 * DK],
                    rec[:, ib * 2 + 1:ib * 2 + 2], None, OP.mult)
                trp = tr_ps.tile([P, P], BF16, name="tr")
                nc.tensor.matmul(
                    trp[:], lhsT=(o_n[:]), rhs=(id_sb[:]),
                    is_transpose=True, start=True, stop=True,
                    skip_group_check=True)
                nc.vector.tensor_copy(oT_blks[pair][ib][:], trp[:])

        # depth-2 software pipeline: sc/exp of slot i+2 issue before the AV
        # block of slot i, so exp latency hides behind PE work
        # ti3's own o_proj for i-block column sc4 only needs pair3's ib=sc4
        # drain (at slot 60+sc4); pull the early columns into the last slots
        tail_oproj = deque()
        if ti == ST - 1:
            for sc4 in range(4):
                for et in range(2):
                    tail_oproj.append((oproj_group, ti, oT_blks, et, sc4))

        nslots = len(slots)
        side_n = len(side)
        popped = 0
        for i in range(nslots):
            phase1(slots[i])
            # side work lands between the scores/exp issue and the
            # exp-blocked AV block (the PE's 4-deep wait queue means work
            # emitted after phase2 can't bypass the stalled AV matmuls),
            # paced evenly across the tile's slots
            want = -(-side_n * (i + 1) // nslots)  # ceil
            while popped < want and side:
                g = side.popleft()
                g[0](*g[1:])
                popped += 1
            phase2(slots[i])
            if ti == ST - 1 and i >= 61 and tail_oproj:
                for _ in range(1 if i == 61 else 2):
                    if tail_oproj:
                        g = tail_oproj.popleft()
                        g[0](*g[1:])
        while side:
            g = side.popleft()
            g[0](*g[1:])
        prev_oproj = (ti, oT_blks)

    # final o_proj (whatever ti3 didn't absorb): alternate between the pp
    # bank and the now-idle scores banks so the tail pipelines
    use_sc = False
    while tail_oproj:
        g = tail_oproj.popleft()
        g[0](*g[1:], use_sc_pool=use_sc, tail=True)
        use_sc = not use_sc


def _host_prep(x, Wq, Wk, Wv, Wo, token_positions):
    """Build the 8 per-core input maps (sharding + layout prep only)."""
    x = np.asarray(x, dtype=np.float32)
    Wq = np.asarray(Wq, dtype=np.float32)
    Wk = np.asarray(Wk, dtype=np.float32)
    Wv = np.asarray(Wv, dtype=np.float32)
    Wo = np.asarray(Wo, dtype=np.float32)
    pos = np.asarray(token_positions)

    half = DK // 2
    inv_freq = THETA ** (-np.arange(half, dtype=np.float64) * 2.0 / DK)
    ang = pos.astype(np.float64)[None, :] * inv_freq[:, None]      # [32, S]
    cos32 = np.cos(ang)
    sin32 = np.sin(ang)
    cosr = np.empty((P, S), dtype=np.float32)
    sinr = np.empty((P, S), dtype=np.float32)
    for p in range(P):
        ip = p % DK
        i = ip % half
        cosr[p] = cos32[i]
        sinr[p] = (-sin32[i]) if ip < half else sin32[i]
    cosr = cosr.astype(ml_dtypes.bfloat16)
    sinr = sinr.astype(ml_dtypes.bfloat16)

    # de-interleave permutation within each head: [evens | odds]
    perm = np.concatenate([np.arange(0, DK, 2), np.arange(1, DK, 2)])

    msk = np.triu(np.ones((P, P), dtype=np.float32)).astype(ml_dtypes.bfloat16)
    ones1 = np.ones((P, 1), dtype=np.float32).astype(ml_dtypes.bfloat16)
    ident = np.eye(P, dtype=np.float32).astype(ml_dtypes.bfloat16)

    WqT = Wq.T  # [d_in, e_out]
    WkT = Wk.T
    WvT = Wv.T
    WoT = Wo.T  # [e_in, d_out]

    in_maps = []
    for core in range(N_CORES):
        b, g = core // HG, core % HG
        cols = np.concatenate(
            [g * CL + h * DK + perm for h in range(H_LOC)])
        plain = slice(g * CL, (g + 1) * CL)
        in_maps.append({
            "xT": np.ascontiguousarray(x[b].T).astype(ml_dtypes.bfloat16),
            "wq": np.ascontiguousarray(WqT[:, cols]).astype(ml_dtypes.bfloat16),
            "wk": np.ascontiguousarray(WkT[:, cols]).astype(ml_dtypes.bfloat16),
            "wv": np.ascontiguousarray(WvT[:, plain]).astype(ml_dtypes.bfloat16),
            "wo": np.ascontiguousarray(WoT[plain, :]).astype(ml_dtypes.bfloat16),
            "cosr": cosr,
            "sinr": sinr,
            "msk": msk,
            "ones1": ones1,
            "ident": ident,
        })
    return in_maps


def kernel(x, Wq, Wk, Wv, Wo, token_positions, _trace=False):
    global LAST_RESULT
    if "nc" not in _CACHE:
        _CACHE["nc"] = _build_program()
    nc = _CACHE["nc"]

    in_maps = _host_prep(x, Wq, Wk, Wv, Wo, token_positions)
    res = run_bass_kernel_spmd(nc, in_maps, core_ids=list(range(N_CORES)),
                               trace=_trace)
    LAST_RESULT = res
    outs = [r["out"] for r in res.results]
    final = np.empty((B, S, D), dtype=np.float32)
    for b in range(B):
        final[b] = outs[b * HG]
        for g in range(1, HG):
            final[b] += outs[b * HG + g]
    return final


# revision 52
# speedup vs baseline: 1.0126x; 1.0011x over previous
"""Trainium2 Bass kernel: causal multi-head self-attention with RoPE.

Problem: B=4, S=2048, D=1024, H=16 heads, dk=64, fp32.
Sharding: 8 cores = (batch b in 0..3) x (head-group g in 0..1, 8 heads each).
Each core computes a partial o_proj output [S, D] for its (b, g); the host
sums the two head-group partials per batch and stacks batches.

Design (vs the 383us fp32r baseline; this version sims at ~258us):
 - Everything downstream of HBM runs on bf16 operands (x, all weights, q/k
   post-RoPE, exp-scores, v, o) with fp32 psum accumulation: bf16 matmuls
   run at 1 cycle/row at ANY moving size, unlike fp32r which pays 4x under
   moving=256, enabling the flipped AV orientation below.  End-to-end L2
   err ~6e-3 (gate 2e-2).
 - AV matmuls are flipped to out[i-block 128 part, dk moving]: lhsT =
   exp-scores tile pt[j, i-block], rhs = v[j, dk].  Cost is charged per
   moving row, so this halves AV PE time vs the [dk+1 part, i moving]
   orientation (full 128-partition output occupancy).  A parallel 1-col
   matmul against a ones vector accumulates the softmax denominators into a
   separate psum bank, making them per-PARTITION scalars; normalize is then
   reciprocal [128,2] + tensor_scalar multiply, and oT for o_proj comes
   from [128,128] PE transposes (53ns each in bf16) into per-(pair,i-block)
   contiguous SBUF lhsT tiles.
 - Scores for both heads of a pair land in one [128,1024] 2-bank psum tile
   and a single bank-spanning exp converts them (185ns/instr access latency
   amortized; deep-diagonal slots split in two to skip the dead gap).
 - PSUM banks (8): 2x scores [128,1024] + 1x AV [128,512] + 1x denom
   [128,32] + 1x transpose + 1x proj/oproj [128,512] (+padding).  Multiple
   accumulation groups share the AV/denom banks with start=True only on the
   bank's first matmul (the backend's pending-zero region is bank-wide;
   later groups' first touch overwrites, then accumulates).
 - RoPE partner values (partition p^32) come from partition-base-shifted
   DVE tensor_copies (engines allow differing in/out partition bases; bf16
   SBUF copies run in 4x DVE mode), replacing 128 SBUF->SBUF DMAs.
 - Weights/x load as single 3-dim-AP DMAs (e.g. [128, 8dt, cols]) instead
   of per-dt strips: ~60 DMA instructions total vs 247 (HWDGE issue and
   SP.SEQ are ~630ns/DMA serial resources).  wq/wk load their cc0 column
   slice and the first 512 cos/sin columns first so projection + RoPE for
   the first attention slot are ready ~6us in.
 - The emission interleaves attention slots (pair, j-tile) with side work:
   projections of later s-tiles (q two tiles ahead, k/v one - k/v of
   s-tile st are only needed from j-tile 4*st of attention tile st) and
   o_proj of the previous tile, paced evenly across each tile's slots and
   placed between the scores issue and the exp-blocked AV block (the PE's
   4-deep wait queue can't bypass a clogged dependent block).  Engine
   assignment of psum-draining copies alternates DVE/ACT by phase to track
   which engine has slack (DVE is the early bottleneck, ACT's exp the late
   one).
"""

import numpy as np
import ml_dtypes
from contextlib import ExitStack

import concourse.bass as bass
import concourse.bacc as bacc
import concourse.mybir as mybir
import concourse.tile as tile
from concourse.ap import AP
from concourse.bass_utils import run_bass_kernel_spmd

B, S, D = 4, 2048, 1024
H_TOT, DK = 16, 64
THETA = 10000.0
N_CORES = 8
HG = 2                  # head groups (cores per batch)
H_LOC = H_TOT // HG     # 8 heads per core
CL = H_LOC * DK         # 512 local channels
P = 128
DT = D // P             # 8 contraction tiles
CC = CL // P            # 4 c-chunks (2 heads each)
ST = S // 512           # 4 s-tiles of 512
F32 = mybir.dt.float32
F32R = mybir.dt.float32r
BF16 = mybir.dt.bfloat16
OP = mybir.AluOpType
AF = mybir.ActivationFunctionType

_CACHE: dict = {}
LAST_RESULT = None  # stashed BassKernelResults for test harness introspection


def _build_program():
    nc = bacc.Bacc(
        "TRN2", target_bir_lowering=False, debug=False,
        num_devices=N_CORES,
    )
    xT = nc.declare_dram_parameter("xT", [D, S], BF16, isOutput=False).ap()
    wq = nc.declare_dram_parameter("wq", [D, CL], BF16, isOutput=False).ap()
    wk = nc.declare_dram_parameter("wk", [D, CL], BF16, isOutput=False).ap()
    wv = nc.declare_dram_parameter("wv", [D, CL], BF16, isOutput=False).ap()
    wo = nc.declare_dram_parameter("wo", [CL, D], BF16, isOutput=False).ap()
    cosr = nc.declare_dram_parameter("cosr", [P, S], BF16, isOutput=False).ap()
    sinr = nc.declare_dram_parameter("sinr", [P, S], BF16, isOutput=False).ap()
    msk = nc.declare_dram_parameter("msk", [P, P], BF16, isOutput=False).ap()
    ones1 = nc.declare_dram_parameter("ones1", [P, 1], BF16, isOutput=False).ap()
    ident = nc.declare_dram_parameter("ident", [P, P], BF16, isOutput=False).ap()
    out = nc.declare_dram_parameter("out", [S, D], F32, isOutput=True).ap()

    with tile.TileContext(nc) as tc:
        with ExitStack() as ctx, nc.allow_low_precision(
                reason="bf16 attention operands; psum accumulation stays fp32"):
            _emit(nc, tc, ctx, xT, wq, wk, wv, wo, cosr, sinr, msk, ones1, ident, out)
    nc.finalize()
    return nc


def _emit(nc, tc, ctx, xT, wq, wk, wv, wo, cosr, sinr, msk, ones1, ident, out):
    consts = ctx.enter_context(tc.tile_pool(name="consts", bufs=1))
    kv_pool = ctx.enter_context(tc.tile_pool(name="kv", bufs=1))
    xt_pool = ctx.enter_context(tc.tile_pool(name="xt", bufs=4))
    qt_pool = ctx.enter_context(tc.tile_pool(name="qt", bufs=4))
    rope_pool = ctx.enter_context(tc.tile_pool(name="rope", bufs=3))
    pt_pool = ctx.enter_context(tc.tile_pool(name="pt", bufs=4))
    on_pool = ctx.enter_context(tc.tile_pool(name="on", bufs=4))
    rec_pool = ctx.enter_context(tc.tile_pool(name="rec", bufs=3))
    ot_pool = ctx.enter_context(tc.tile_pool(name="ot", bufs=4))
    ob_pool = ctx.enter_context(tc.tile_pool(name="ob", bufs=3))
    sc_ps = ctx.enter_context(tc.tile_pool(name="scps", bufs=2, space="PSUM"))
    av_ps = ctx.enter_context(tc.tile_pool(name="avps", bufs=1, space="PSUM"))
    ad_ps = ctx.enter_context(tc.tile_pool(name="adps", bufs=1, space="PSUM"))
    tr_ps = ctx.enter_context(tc.tile_pool(name="trps", bufs=1, space="PSUM"))
    pp_ps = ctx.enter_context(tc.tile_pool(name="pp", bufs=1, space="PSUM"))

    # ---- resident constants --------------------------------------------
    wq_sb = consts.tile([P, DT, CL], BF16, name="wq_sb")
    wk_sb = consts.tile([P, DT, CL], BF16, name="wk_sb")
    wv_sb = consts.tile([P, DT, CL], BF16, name="wv_sb")
    wo_sb = consts.tile([P, CC, D], BF16, name="wo_sb")
    cos_sb = consts.tile([P, S], BF16, name="cos_sb")
    sin_sb = consts.tile([P, S], BF16, name="sin_sb")
    msk_sb = consts.tile([P, P], BF16, name="msk_sb")
    ones_sb = consts.tile([P, 1], BF16, name="ones_sb")
    id_sb = consts.tile([P, P], BF16, name="id_sb")
    kT_t = [kv_pool.tile([P, CC, 512], BF16, name=f"kT{st}") for st in range(ST)]
    vv_t = [kv_pool.tile([P, H_LOC, DK], BF16, name=f"vv{j}")
            for j in range(S // P)]

    def w_slice_dma(dst_sb, w_ap, c0, c1):
        # dst_sb[:, dt, c0:c1] <- w[dt*128+p, c0:c1] in ONE 3-dim DMA
        src = AP(w_ap.tensor, w_ap.offset + c0, [[CL, P], [CL * P, DT], [1, c1 - c0]])
        nc.sync.dma_start(dst_sb[:, :, c0:c1], src)

    def load_consts():
        # ordered for fast compute start: first proj group is q(cc0, st0)
        w_slice_dma(wq_sb, wq, 0, P)
        w_slice_dma(wk_sb, wk, 0, P)
        for dt in range(DT):  # xt st0 per-dt so q0/k0 accumulation chases arrivals
            nc.sync.dma_start(
                xt0[:, dt, :], xT[dt * P:(dt + 1) * P, 0:512])
        # first 512 cols of cos/sin unblock st0's RoPE ~5us earlier
        nc.sync.dma_start(cos_sb[:, 0:512], cosr[:, 0:512])
        nc.sync.dma_start(sin_sb[:, 0:512], sinr[:, 0:512])
        for dt in range(DT):  # per-dt so v0's accumulation chases arrivals
            nc.sync.dma_start(
                wv_sb[:, dt, :], wv[dt * P:(dt + 1) * P, :])
        nc.sync.dma_start(cos_sb[:, 512:S], cosr[:, 512:S])
        nc.sync.dma_start(sin_sb[:, 512:S], sinr[:, 512:S])
        nc.sync.dma_start(msk_sb[:], msk)
        nc.sync.dma_start(ones_sb[:], ones1)
        nc.sync.dma_start(id_sb[:], ident)
        w_slice_dma(wq_sb, wq, P, CL)
        w_slice_dma(wk_sb, wk, P, CL)
        wo_src = AP(wo.tensor, wo.offset, [[D, P], [D * P, CC], [1, D]])
        nc.sync.dma_start(wo_sb[:], wo_src)

    from collections import deque

    def load_st(st):
        s0 = st * 512
        xt = xt_pool.tile([P, DT, 512], BF16, name="xt")
        src = AP(xT.tensor, xT.offset + s0, [[S, P], [S * P, DT], [1, 512]])
        nc.sync.dma_start(xt[:], src)
        qTs = qt_pool.tile([P, CC, 512], BF16, name="qTs")
        return xt, qTs

    def qk_group(st, cc, which, ctx_tiles):
        xt, qTs = ctx_tiles
        s0 = st * 512
        w_sb = wq_sb if which == "q" else wk_sb
        dst = qTs[:, cc, :] if which == "q" else kT_t[st][:, cc, :]
        ps = pp_ps.tile([P, 512], F32, name="pp")
        for dt in range(DT):
            nc.tensor.matmul(
                ps[:], lhsT=(w_sb[:, dt, cc * P:(cc + 1) * P]),
                rhs=(xt[:, dt, :]), start=(dt == 0), stop=(dt == DT - 1))
        qraw = rope_pool.tile([P, 512], BF16, name="qraw")
        # psum->sbuf copy on ACT for early s-tiles (ACT idles while DVE is
        # the early bottleneck); DVE for late ones (ACT saturates on exp)
        if st <= 1:
            nc.scalar.copy(qraw[:], ps[:])
        else:
            nc.vector.tensor_copy(qraw[:], ps[:])
        # partner view (partition p ^ 32) via partition-base-shifted DVE
        # copies (bf16 SBUF copies run in 4x DVE mode)
        qsh = rope_pool.tile([P, 512], BF16, name="qsh")
        for blk in range(4):
            p0 = blk * 32
            q0 = p0 ^ 32
            nc.vector.tensor_copy(qsh[p0:p0 + 32, :], qraw[q0:q0 + 32, :])
        tcos = rope_pool.tile([P, 512], BF16, name="tcos")
        nc.gpsimd.tensor_tensor(tcos[:], qraw[:], cos_sb[:, s0:s0 + 512], OP.mult)
        nc.vector.tensor_tensor(dst, qsh[:], sin_sb[:, s0:s0 + 512], OP.mult)
        nc.vector.tensor_tensor(dst, dst, tcos[:], OP.add)

    def v_group(st, sc, ctx_tiles):
        xt = ctx_tiles[0]
        gsc = st * 4 + sc
        ps = pp_ps.tile([P, 512], F32, name="pp")
        for dt in range(DT):
            nc.tensor.matmul(
                ps[:], lhsT=(xt[:, dt, sc * P:(sc + 1) * P]),
                rhs=(wv_sb[:, dt, :]), start=(dt == 0), stop=(dt == DT - 1))
        if st <= 1:
            nc.scalar.copy(
                vv_t[gsc][:], ps.rearrange("p (h c) -> p h c", c=DK))
        else:
            nc.vector.tensor_copy(
                vv_t[gsc][:], ps.rearrange("p (h c) -> p h c", c=DK))

    def oproj_group(ti, oT_blks, et, sc4, use_sc_pool=False, tail=False):
        s0 = ti * 512
        if use_sc_pool:
            ps = sc_ps.tile([P, 512], F32, name="sc")
        else:
            ps = pp_ps.tile([P, 512], F32, name="pp")
        for cc in range(CC):
            nc.tensor.matmul(
                ps[:], lhsT=(oT_blks[cc][sc4][:]),
                rhs=(wo_sb[:, cc, et * 512:(et + 1) * 512]),
                start=(cc == 0), stop=(cc == CC - 1))
        osb = ob_pool.tile([P, 512], F32, name="osb")
        rows = slice(s0 + sc4 * P, s0 + (sc4 + 1) * P)
        if tail:
            # epilogue: ACT and HWDGE are idle - split the copy and the
            # out-DMA in halves so the final chain pipelines
            nc.vector.tensor_copy(osb[:, 0:256], ps[:, 0:256])
            nc.sync.dma_start(
                out[rows, et * 512:et * 512 + 256], osb[:, 0:256])
            nc.scalar.copy(osb[:, 256:512], ps[:, 256:512])
            nc.sync.dma_start(
                out[rows, et * 512 + 256:(et + 1) * 512], osb[:, 256:512])
        else:
            # psum->sbuf staging on DVE (ACT stays exp-only; GPSIMD cannot
            # read PSUM)
            nc.vector.tensor_copy(osb[:], ps[:])
            nc.sync.dma_start(
                out[rows, et * 512:(et + 1) * 512], osb[:])

    # prologue: minimal inline projections (q-cc0, k-cc0, v0); the rest of
    # st0's projections go through the global side queue so attention(ti=0)
    # starts ~7us in
    xt0 = xt_pool.tile([P, DT, 512], BF16, name="xt")
    qTs0 = qt_pool.tile([P, CC, 512], BF16, name="qTs")
    ctx0 = (xt0, qTs0)
    load_consts()
    qk_group(0, 0, "q", ctx0)
    qk_group(0, 0, "k", ctx0)
    v_group(0, 0, ctx0)
    st_tiles = {0: ctx0}

    # ------------------------------------------------------------------
    # Global side-work queue.  k/v projections of s-tile st are legal to
    # run DURING attention(ti=st) up to jt=4*st; q projections of st must
    # finish before ti=st starts.  o_proj(ti) runs during ti+1.
    # ------------------------------------------------------------------
    side = deque()
    deferred_oproj = None

    def seed_side_for_ti(ti):
        # called at ti start: queue o_proj(ti-1), then k/v(st=ti) early-need
        # groups were queued at ti-1... layout below.
        pass

    prev_oproj = None
    for ti in range(ST):
        qTs = st_tiles[ti][1]
        # deferred k/v of THIS s-tile first (needed from jt=4*ti onward),
        # then o_proj of the previous tile, then next tile's q projections.
        if ti >= 1:
            ctxn = st_tiles[ti]
            side.appendleft((qk_group, ti, 0, "k", ctxn))
            pos = 1
            for sc in range(4):
                side.insert(pos, (v_group, ti, sc, ctxn))
                pos += 1
            for cc in range(1, CC):
                side.insert(pos, (qk_group, ti, cc, "k", ctxn))
                pos += 1
        if deferred_oproj is not None:
            pti, poT = deferred_oproj
            for sc4 in range(4):
                side.append((oproj_group, pti, poT, 1, sc4))
            deferred_oproj = None
        if prev_oproj is not None:
            pti, poT = prev_oproj
            for et in range(2):
                for sc4 in range(4):
                    if pti == 1 and et == 1:
                        continue  # deferred into ti3
                    side.append((oproj_group, pti, poT, et, sc4))
            if pti == 1:
                deferred_oproj = (pti, poT)
            prev_oproj = None
        if ti + 1 < ST and ti + 1 not in st_tiles:
            nxt = load_st(ti + 1)
            st_tiles[ti + 1] = nxt
            if ti + 1 == 1:
                # remaining st0 groups first (attention ti0 needs them)
                side.extend([(v_group, 0, sc, ctx0) for sc in (1, 2, 3)])
                side.extend([(qk_group, 0, cc, w, ctx0)
                             for cc in (1, 2, 3) for w in ("q", "k")])
            if ti == 0:
                # q projections of st1 (must complete before ti1)
                side.extend([(qk_group, ti + 1, cc, "q", nxt)
                             for cc in range(CC)])
        if ti + 2 < ST and ti + 2 not in st_tiles:
            nxt2 = load_st(ti + 2)
            st_tiles[ti + 2] = nxt2
            # q projections of st+2 run in THIS tile's tail (they only need
            # xt(st+2), an input) so ti+1 keeps its slots for k/v + o_proj
            side.extend([(qk_group, ti + 2, cc, "q", nxt2)
                         for cc in range(CC)])

        oT_blks = [[ot_pool.tile([P, P], BF16, name=f"oTb_{pr}_{b}")
                    for b in range(4)] for pr in range(CC)]
        ad = ad_ps.tile([P, 32], F32, name="ad")
        njt = 4 * (ti + 1)
        slots = [(pair, jt) for pair in range(CC) for jt in range(njt)]
        state = {"ad_started": False, "av": None, "av_started": False,
                 "rec": None, "pt": {}, "sc": {}}

        def phase1(slot):
            pair, jt = slot
            r = max(0, (jt - 4 * ti) * P)
            kslc = kT_t[jt // 4][:, pair, (jt % 4) * P:(jt % 4) * P + P]
            scb = sc_ps.tile([P, 1024], F32, name="sc")
            nc.tensor.matmul(
                scb[:, r:512], lhsT=(kslc[0:DK]),
                rhs=(qTs[0:DK, pair, r:512]), start=True, stop=True)
            nc.tensor.matmul(
                scb[:, 512 + r:1024], lhsT=(kslc[DK:P]),
                rhs=(qTs[DK:P, pair, r:512]), start=True, stop=True,
                skip_group_check=True)
            pt = pt_pool.tile([P, 1024], BF16, name="pt")
            if r >= 256:
                # deep-diagonal slots: two exps beat paying for the dead gap
                nc.scalar.activation(
                    pt[:, r:512], scb[:, r:512], AF.Exp, scale=0.125)
                nc.scalar.activation(
                    pt[:, 512 + r:1024], scb[:, 512 + r:1024], AF.Exp,
                    scale=0.125)
            else:
                # one bank-spanning exp covering both heads' ranges; the gap
                # cols [512:512+r) hold stale psum whose exp is never read
                nc.scalar.activation(
                    pt[:, r:1024], scb[:, r:1024], AF.Exp, scale=0.125)
            state["sc"][slot] = scb
            if jt >= 4 * ti:
                nc.vector.tensor_tensor(
                    pt[:, r:r + P], pt[:, r:r + P], msk_sb[:], OP.mult)
                nc.vector.tensor_tensor(
                    pt[:, 512 + r:512 + r + P], pt[:, 512 + r:512 + r + P],
                    msk_sb[:], OP.mult)
            state["pt"][slot] = pt

        def phase2(slot):
            pair, jt = slot
            pt = state["pt"].pop(slot)
            state["sc"].pop(slot)
            if jt == 0:
                state["av"] = av_ps.tile([P, 512], F32, name="av")
                state["av_started"] = False
                state["rec"] = rec_pool.tile([P, 8], F32, name="rec")
            av, rec = state["av"], state["rec"]
            ib0 = max(0, jt - 4 * ti)
            # h0 AV matmuls first (they only wait on the h0 exp), then h1
            for hh in range(2):
                for ib in range(ib0, 4):
                    last = (jt == 4 * ti + ib)
                    h = 2 * pair + hh
                    lhs = (pt[:, ib * P:ib * P + P] if hh == 0
                           else pt[:, 512 + ib * P:512 + ib * P + P])
                    nc.tensor.matmul(
                        av[:, ib * P + hh * DK:ib * P + hh * DK + DK],
                        lhsT=(lhs), rhs=(vv_t[jt][:, h, :]),
                        start=(not state["av_started"]), stop=last,
                        skip_group_check=True)
                    state["av_started"] = True
                    dc = pair * 8 + ib * 2 + hh
                    nc.tensor.matmul(
                        ad[:, dc:dc + 1], lhsT=(lhs), rhs=(ones_sb[:]),
                        start=(not state["ad_started"]), stop=last,
                        skip_group_check=True)
                    state["ad_started"] = True
            ibd = jt - 4 * ti
            if 0 <= ibd <= 3:
                ib = ibd
                # drain (pair, ib): per-partition normalize, PE transpose,
                # copy out to the SBUF lhsT tile for o_proj
                c2 = pair * 8 + ib * 2
                nc.vector.reciprocal(
                    rec[:, ib * 2:ib * 2 + 2], ad[:, c2:c2 + 2])
                o_n = on_pool.tile([P, P], BF16, name="o_n")
                nc.vector.tensor_scalar(
                    o_n[:, 0:DK],
                    av[:, ib * P:ib * P + DK],
                    rec[:, ib * 2:ib * 2 + 1], None, OP.mult)
                nc.vector.tensor_scalar(
                    o_n[:, DK:2 * DK],
                    av[:, ib * P + DK:ib * P + 2 * DK],
                    rec[:, ib * 2 + 1:ib * 2 + 2], None, OP.mult)
                trp = tr_ps.tile([P, P], BF16, name="tr")
                nc.tensor.matmul(
                    trp[:], lhsT=(o_n[:]), rhs=(id_sb[:]),
                    is_transpose=True, start=True, stop=True,
                    skip_group_check=True)
                nc.vector.tensor_copy(oT_blks[pair][ib][:], trp[:])

        # depth-2 software pipeline: sc/exp of slot i+2 issue before the AV
        # block of slot i, so exp latency hides behind PE work
        # ti3's own o_proj for i-block column sc4 only needs pair3's ib=sc4
        # drain (at slot 60+sc4); pull the early columns into the last slots
        tail_oproj = deque()
        if ti == ST - 1:
            for sc4 in range(4):
                for et in range(2):
                    tail_oproj.append((oproj_group, ti, oT_blks, et, sc4))

        nslots = len(slots)
        side_n = len(side)
        popped = 0
        for i in range(nslots):
            phase1(slots[i])
            # side work lands between the scores/exp issue and the
            # exp-blocked AV block (the PE's 4-deep wait queue means work
            # emitted after phase2 can't bypass the stalled AV matmuls),
            # paced evenly across the tile's slots
            want = -(-side_n * (i + 1) // nslots)  # ceil
            while popped < want and side:
                g = side.popleft()
                g[0](*g[1:])
                popped += 1
            phase2(slots[i])
            if ti == ST - 1 and i >= 61 and tail_oproj:
                for _ in range(2):
                    if tail_oproj:
                        g = tail_oproj.popleft()
                        g[0](*g[1:])
        while side:
            g = side.popleft()
            g[0](*g[1:])
        prev_oproj = (ti, oT_blks)

    # final o_proj (whatever ti3 didn't absorb): alternate between the pp
    # bank and the now-idle scores banks so the tail pipelines
    use_sc = False
    while tail_oproj:
        g = tail_oproj.popleft()
        g[0](*g[1:], use_sc_pool=use_sc, tail=True)
        use_sc = not use_sc


def _host_prep(x, Wq, Wk, Wv, Wo, token_positions):
    """Build the 8 per-core input maps (sharding + layout prep only)."""
    x = np.asarray(x, dtype=np.float32)
    Wq = np.asarray(Wq, dtype=np.float32)
    Wk = np.asarray(Wk, dtype=np.float32)
    Wv = np.asarray(Wv, dtype=np.float32)
    Wo = np.asarray(Wo, dtype=np.float32)
    pos = np.asarray(token_positions)

    half = DK // 2
    inv_freq = THETA ** (-np.arange(half, dtype=np.float64) * 2.0 / DK)
    ang = pos.astype(np.float64)[None, :] * inv_freq[:, None]      # [32, S]
    cos32 = np.cos(ang)
    sin32 = np.sin(ang)
    cosr = np.empty((P, S), dtype=np.float32)
    sinr = np.empty((P, S), dtype=np.float32)
    for p in range(P):
        ip = p % DK
        i = ip % half
        cosr[p] = cos32[i]
        sinr[p] = (-sin32[i]) if ip < half else sin32[i]
    cosr = cosr.astype(ml_dtypes.bfloat16)
    sinr = sinr.astype(ml_dtypes.bfloat16)

    # de-interleave permutation within each head: [evens | odds]
    perm = np.concatenate([np.arange(0, DK, 2), np.arange(1, DK, 2)])

    msk = np.triu(np.ones((P, P), dtype=np.float32)).astype(ml_dtypes.bfloat16)
    ones1 = np.ones((P, 1), dtype=np.float32).astype(ml_dtypes.bfloat16)
    ident = np.eye(P, dtype=np.float32).astype(ml_dtypes.bfloat16)

    WqT = Wq.T  # [d_in, e_out]
    WkT = Wk.T
    WvT = Wv.T
    WoT = Wo.T  # [e_in, d_out]

    in_maps = []
    for core in range(N_CORES):
        b, g = core // HG, core % HG
        cols = np.concatenate(
            [g * CL + h * DK + perm for h in range(H_LOC)])
        plain = slice(g * CL, (g + 1) * CL)
        in_maps.append({
            "xT": np.ascontiguousarray(x[b].T).astype(ml_dtypes.bfloat16),
            "wq": np.ascontiguousarray(WqT[:, cols]).astype(ml_dtypes.bfloat16),
            "wk": np.ascontiguousarray(WkT[:, cols]).astype(ml_dtypes.bfloat16),
            "wv": np.ascontiguousarray(WvT[:, plain]).astype(ml_dtypes.bfloat16),
            "wo": np.ascontiguousarray(WoT[plain, :]).astype(ml_dtypes.bfloat16),
            "cosr": cosr,
            "sinr": sinr,
            "msk": msk,
            "ones1": ones1,
            "ident": ident,
        })
    return in_maps


def kernel(x, Wq, Wk, Wv, Wo, token_positions, _trace=False):
    global LAST_RESULT
    if "nc" not in _CACHE:
        _CACHE["nc"] = _build_program()
    nc = _CACHE["nc"]

    in_maps = _host_prep(x, Wq, Wk, Wv, Wo, token_positions)
    res = run_bass_kernel_spmd(nc, in_maps, core_ids=list(range(N_CORES)),
                               trace=_trace)
    LAST_RESULT = res
    outs = [r["out"] for r in res.results]
    final = np.empty((B, S, D), dtype=np.float32)
    for b in range(B):
        final[b] = outs[b * HG]
        for g in range(1, HG):
            final[b] += outs[b * HG + g]
    return final


# revision 53
# speedup vs baseline: 1.0172x; 1.0045x over previous
"""Trainium2 Bass kernel: causal multi-head self-attention with RoPE.

Problem: B=4, S=2048, D=1024, H=16 heads, dk=64, fp32.
Sharding: 8 cores = (batch b in 0..3) x (head-group g in 0..1, 8 heads each).
Each core computes a partial o_proj output [S, D] for its (b, g); the host
sums the two head-group partials per batch and stacks batches.

Design (vs the 383us fp32r baseline; this version sims at ~258us):
 - Everything downstream of HBM runs on bf16 operands (x, all weights, q/k
   post-RoPE, exp-scores, v, o) with fp32 psum accumulation: bf16 matmuls
   run at 1 cycle/row at ANY moving size, unlike fp32r which pays 4x under
   moving=256, enabling the flipped AV orientation below.  End-to-end L2
   err ~6e-3 (gate 2e-2).
 - AV matmuls are flipped to out[i-block 128 part, dk moving]: lhsT =
   exp-scores tile pt[j, i-block], rhs = v[j, dk].  Cost is charged per
   moving row, so this halves AV PE time vs the [dk+1 part, i moving]
   orientation (full 128-partition output occupancy).  A parallel 1-col
   matmul against a ones vector accumulates the softmax denominators into a
   separate psum bank, making them per-PARTITION scalars; normalize is then
   reciprocal [128,2] + tensor_scalar multiply, and oT for o_proj comes
   from [128,128] PE transposes (53ns each in bf16) into per-(pair,i-block)
   contiguous SBUF lhsT tiles.
 - Scores for both heads of a pair land in one [128,1024] 2-bank psum tile
   and a single bank-spanning exp converts them (185ns/instr access latency
   amortized; deep-diagonal slots split in two to skip the dead gap).
 - PSUM banks (8): 2x scores [128,1024] + 1x AV [128,512] + 1x denom
   [128,32] + 1x transpose + 1x proj/oproj [128,512] (+padding).  Multiple
   accumulation groups share the AV/denom banks with start=True only on the
   bank's first matmul (the backend's pending-zero region is bank-wide;
   later groups' first touch overwrites, then accumulates).
 - RoPE partner values (partition p^32) come from partition-base-shifted
   DVE tensor_copies (engines allow differing in/out partition bases; bf16
   SBUF copies run in 4x DVE mode), replacing 128 SBUF->SBUF DMAs.
 - Weights/x load as single 3-dim-AP DMAs (e.g. [128, 8dt, cols]) instead
   of per-dt strips: ~60 DMA instructions total vs 247 (HWDGE issue and
   SP.SEQ are ~630ns/DMA serial resources).  wq/wk load their cc0 column
   slice and the first 512 cos/sin columns first so projection + RoPE for
   the first attention slot are ready ~6us in.
 - The emission interleaves attention slots (pair, j-tile) with side work:
   projections of later s-tiles (q two tiles ahead, k/v one - k/v of
   s-tile st are only needed from j-tile 4*st of attention tile st) and
   o_proj of the previous tile, paced evenly across each tile's slots and
   placed between the scores issue and the exp-blocked AV block (the PE's
   4-deep wait queue can't bypass a clogged dependent block).  Engine
   assignment of psum-draining copies alternates DVE/ACT by phase to track
   which engine has slack (DVE is the early bottleneck, ACT's exp the late
   one).
"""

import numpy as np
import ml_dtypes
from contextlib import ExitStack

import concourse.bass as bass
import concourse.bacc as bacc
import concourse.mybir as mybir
import concourse.tile as tile
from concourse.ap import AP
from concourse.bass_utils import run_bass_kernel_spmd

B, S, D = 4, 2048, 1024
H_TOT, DK = 16, 64
THETA = 10000.0
N_CORES = 8
HG = 2                  # head groups (cores per batch)
H_LOC = H_TOT // HG     # 8 heads per core
CL = H_LOC * DK         # 512 local channels
P = 128
DT = D // P             # 8 contraction tiles
CC = CL // P            # 4 c-chunks (2 heads each)
ST = S // 512           # 4 s-tiles of 512
F32 = mybir.dt.float32
F32R = mybir.dt.float32r
BF16 = mybir.dt.bfloat16
OP = mybir.AluOpType
AF = mybir.ActivationFunctionType

_CACHE: dict = {}
LAST_RESULT = None  # stashed BassKernelResults for test harness introspection


def _build_program():
    nc = bacc.Bacc(
        "TRN2", target_bir_lowering=False, debug=False,
        num_devices=N_CORES,
    )
    xT = nc.declare_dram_parameter("xT", [D, S], BF16, isOutput=False).ap()
    wq = nc.declare_dram_parameter("wq", [D, CL], BF16, isOutput=False).ap()
    wk = nc.declare_dram_parameter("wk", [D, CL], BF16, isOutput=False).ap()
    wv = nc.declare_dram_parameter("wv", [D, CL], BF16, isOutput=False).ap()
    wo = nc.declare_dram_parameter("wo", [CL, D], BF16, isOutput=False).ap()
    cosr = nc.declare_dram_parameter("cosr", [P, S], BF16, isOutput=False).ap()
    sinr = nc.declare_dram_parameter("sinr", [P, S], BF16, isOutput=False).ap()
    msk = nc.declare_dram_parameter("msk", [P, P], BF16, isOutput=False).ap()
    ones1 = nc.declare_dram_parameter("ones1", [P, 1], BF16, isOutput=False).ap()
    ident = nc.declare_dram_parameter("ident", [P, P], BF16, isOutput=False).ap()
    out = nc.declare_dram_parameter("out", [S, D], F32, isOutput=True).ap()

    with tile.TileContext(nc) as tc:
        with ExitStack() as ctx, nc.allow_low_precision(
                reason="bf16 attention operands; psum accumulation stays fp32"):
            _emit(nc, tc, ctx, xT, wq, wk, wv, wo, cosr, sinr, msk, ones1, ident, out)
    nc.finalize()
    return nc


def _emit(nc, tc, ctx, xT, wq, wk, wv, wo, cosr, sinr, msk, ones1, ident, out):
    consts = ctx.enter_context(tc.tile_pool(name="consts", bufs=1))
    kv_pool = ctx.enter_context(tc.tile_pool(name="kv", bufs=1))
    xt_pool = ctx.enter_context(tc.tile_pool(name="xt", bufs=4))
    qt_pool = ctx.enter_context(tc.tile_pool(name="qt", bufs=4))
    rope_pool = ctx.enter_context(tc.tile_pool(name="rope", bufs=4))
    pt_pool = ctx.enter_context(tc.tile_pool(name="pt", bufs=5))
    on_pool = ctx.enter_context(tc.tile_pool(name="on", bufs=5))
    rec_pool = ctx.enter_context(tc.tile_pool(name="rec", bufs=3))
    ot_pool = ctx.enter_context(tc.tile_pool(name="ot", bufs=4))
    ob_pool = ctx.enter_context(tc.tile_pool(name="ob", bufs=4))
    sc_ps = ctx.enter_context(tc.tile_pool(name="scps", bufs=2, space="PSUM"))
    av_ps = ctx.enter_context(tc.tile_pool(name="avps", bufs=1, space="PSUM"))
    ad_ps = ctx.enter_context(tc.tile_pool(name="adps", bufs=1, space="PSUM"))
    tr_ps = ctx.enter_context(tc.tile_pool(name="trps", bufs=1, space="PSUM"))
    pp_ps = ctx.enter_context(tc.tile_pool(name="pp", bufs=1, space="PSUM"))

    # ---- resident constants --------------------------------------------
    wq_sb = consts.tile([P, DT, CL], BF16, name="wq_sb")
    wk_sb = consts.tile([P, DT, CL], BF16, name="wk_sb")
    wv_sb = consts.tile([P, DT, CL], BF16, name="wv_sb")
    wo_sb = consts.tile([P, CC, D], BF16, name="wo_sb")
    cos_sb = consts.tile([P, S], BF16, name="cos_sb")
    sin_sb = consts.tile([P, S], BF16, name="sin_sb")
    msk_sb = consts.tile([P, P], BF16, name="msk_sb")
    ones_sb = consts.tile([P, 1], BF16, name="ones_sb")
    id_sb = consts.tile([P, P], BF16, name="id_sb")
    kT_t = [kv_pool.tile([P, CC, 512], BF16, name=f"kT{st}") for st in range(ST)]
    vv_t = [kv_pool.tile([P, H_LOC, DK], BF16, name=f"vv{j}")
            for j in range(S // P)]

    def w_slice_dma(dst_sb, w_ap, c0, c1):
        # dst_sb[:, dt, c0:c1] <- w[dt*128+p, c0:c1] in ONE 3-dim DMA
        src = AP(w_ap.tensor, w_ap.offset + c0, [[CL, P], [CL * P, DT], [1, c1 - c0]])
        nc.sync.dma_start(dst_sb[:, :, c0:c1], src)

    def load_consts():
        # ordered for fast compute start: first proj group is q(cc0, st0)
        w_slice_dma(wq_sb, wq, 0, P)
        w_slice_dma(wk_sb, wk, 0, P)
        for dt in range(DT):  # xt st0 per-dt so q0/k0 accumulation chases arrivals
            nc.sync.dma_start(
                xt0[:, dt, :], xT[dt * P:(dt + 1) * P, 0:512])
        # first 512 cols of cos/sin unblock st0's RoPE ~5us earlier
        nc.sync.dma_start(cos_sb[:, 0:512], cosr[:, 0:512])
        nc.sync.dma_start(sin_sb[:, 0:512], sinr[:, 0:512])
        for dt in range(DT):  # per-dt so v0's accumulation chases arrivals
            nc.sync.dma_start(
                wv_sb[:, dt, :], wv[dt * P:(dt + 1) * P, :])
        nc.sync.dma_start(cos_sb[:, 512:S], cosr[:, 512:S])
        nc.sync.dma_start(sin_sb[:, 512:S], sinr[:, 512:S])
        nc.sync.dma_start(msk_sb[:], msk)
        nc.sync.dma_start(ones_sb[:], ones1)
        nc.sync.dma_start(id_sb[:], ident)
        w_slice_dma(wq_sb, wq, P, CL)
        w_slice_dma(wk_sb, wk, P, CL)
        wo_src = AP(wo.tensor, wo.offset, [[D, P], [D * P, CC], [1, D]])
        nc.sync.dma_start(wo_sb[:], wo_src)

    from collections import deque

    def load_st(st):
        s0 = st * 512
        xt = xt_pool.tile([P, DT, 512], BF16, name="xt")
        src = AP(xT.tensor, xT.offset + s0, [[S, P], [S * P, DT], [1, 512]])
        nc.sync.dma_start(xt[:], src)
        qTs = qt_pool.tile([P, CC, 512], BF16, name="qTs")
        return xt, qTs

    def qk_group(st, cc, which, ctx_tiles):
        xt, qTs = ctx_tiles
        s0 = st * 512
        w_sb = wq_sb if which == "q" else wk_sb
        dst = qTs[:, cc, :] if which == "q" else kT_t[st][:, cc, :]
        ps = pp_ps.tile([P, 512], F32, name="pp")
        for dt in range(DT):
            nc.tensor.matmul(
                ps[:], lhsT=(w_sb[:, dt, cc * P:(cc + 1) * P]),
                rhs=(xt[:, dt, :]), start=(dt == 0), stop=(dt == DT - 1))
        qraw = rope_pool.tile([P, 512], BF16, name="qraw")
        # psum->sbuf copy on ACT for early s-tiles (ACT idles while DVE is
        # the early bottleneck); DVE for late ones (ACT saturates on exp)
        if st <= 1:
            nc.scalar.copy(qraw[:], ps[:])
        else:
            nc.vector.tensor_copy(qraw[:], ps[:])
        # partner view (partition p ^ 32) via partition-base-shifted DVE
        # copies (bf16 SBUF copies run in 4x DVE mode)
        qsh = rope_pool.tile([P, 512], BF16, name="qsh")
        for blk in range(4):
            p0 = blk * 32
            q0 = p0 ^ 32
            nc.vector.tensor_copy(qsh[p0:p0 + 32, :], qraw[q0:q0 + 32, :])
        tcos = rope_pool.tile([P, 512], BF16, name="tcos")
        nc.gpsimd.tensor_tensor(tcos[:], qraw[:], cos_sb[:, s0:s0 + 512], OP.mult)
        nc.vector.tensor_tensor(dst, qsh[:], sin_sb[:, s0:s0 + 512], OP.mult)
        nc.vector.tensor_tensor(dst, dst, tcos[:], OP.add)

    def v_group(st, sc, ctx_tiles):
        xt = ctx_tiles[0]
        gsc = st * 4 + sc
        ps = pp_ps.tile([P, 512], F32, name="pp")
        for dt in range(DT):
            nc.tensor.matmul(
                ps[:], lhsT=(xt[:, dt, sc * P:(sc + 1) * P]),
                rhs=(wv_sb[:, dt, :]), start=(dt == 0), stop=(dt == DT - 1))
        if st <= 1:
            nc.scalar.copy(
                vv_t[gsc][:], ps.rearrange("p (h c) -> p h c", c=DK))
        else:
            nc.vector.tensor_copy(
                vv_t[gsc][:], ps.rearrange("p (h c) -> p h c", c=DK))

    def oproj_group(ti, oT_blks, et, sc4, use_sc_pool=False, tail=False):
        s0 = ti * 512
        if use_sc_pool:
            ps = sc_ps.tile([P, 512], F32, name="sc")
        else:
            ps = pp_ps.tile([P, 512], F32, name="pp")
        for cc in range(CC):
            nc.tensor.matmul(
                ps[:], lhsT=(oT_blks[cc][sc4][:]),
                rhs=(wo_sb[:, cc, et * 512:(et + 1) * 512]),
                start=(cc == 0), stop=(cc == CC - 1))
        osb = ob_pool.tile([P, 512], F32, name="osb")
        rows = slice(s0 + sc4 * P, s0 + (sc4 + 1) * P)
        if tail:
            # epilogue: ACT and HWDGE are idle - split the copy and the
            # out-DMA in halves so the final chain pipelines
            nc.vector.tensor_copy(osb[:, 0:256], ps[:, 0:256])
            nc.sync.dma_start(
                out[rows, et * 512:et * 512 + 256], osb[:, 0:256])
            nc.scalar.copy(osb[:, 256:512], ps[:, 256:512])
            nc.sync.dma_start(
                out[rows, et * 512 + 256:(et + 1) * 512], osb[:, 256:512])
        else:
            # psum->sbuf staging on DVE (ACT stays exp-only; GPSIMD cannot
            # read PSUM)
            nc.vector.tensor_copy(osb[:], ps[:])
            nc.sync.dma_start(
                out[rows, et * 512:(et + 1) * 512], osb[:])

    # prologue: minimal inline projections (q-cc0, k-cc0, v0); the rest of
    # st0's projections go through the global side queue so attention(ti=0)
    # starts ~7us in
    xt0 = xt_pool.tile([P, DT, 512], BF16, name="xt")
    qTs0 = qt_pool.tile([P, CC, 512], BF16, name="qTs")
    ctx0 = (xt0, qTs0)
    load_consts()
    qk_group(0, 0, "q", ctx0)
    qk_group(0, 0, "k", ctx0)
    v_group(0, 0, ctx0)
    st_tiles = {0: ctx0}

    # ------------------------------------------------------------------
    # Global side-work queue.  k/v projections of s-tile st are legal to
    # run DURING attention(ti=st) up to jt=4*st; q projections of st must
    # finish before ti=st starts.  o_proj(ti) runs during ti+1.
    # ------------------------------------------------------------------
    side = deque()
    deferred_oproj = None

    def seed_side_for_ti(ti):
        # called at ti start: queue o_proj(ti-1), then k/v(st=ti) early-need
        # groups were queued at ti-1... layout below.
        pass

    prev_oproj = None
    for ti in range(ST):
        qTs = st_tiles[ti][1]
        # deferred k/v of THIS s-tile first (needed from jt=4*ti onward),
        # then o_proj of the previous tile, then next tile's q projections.
        if ti >= 1:
            ctxn = st_tiles[ti]
            side.appendleft((qk_group, ti, 0, "k", ctxn))
            pos = 1
            for sc in range(4):
                side.insert(pos, (v_group, ti, sc, ctxn))
                pos += 1
            for cc in range(1, CC):
                side.insert(pos, (qk_group, ti, cc, "k", ctxn))
                pos += 1
        if deferred_oproj is not None:
            pti, poT = deferred_oproj
            for sc4 in range(4):
                side.append((oproj_group, pti, poT, 1, sc4))
            deferred_oproj = None
        if prev_oproj is not None:
            pti, poT = prev_oproj
            for et in range(2):
                for sc4 in range(4):
                    if pti == 1 and et == 1:
                        continue  # deferred into ti3
                    side.append((oproj_group, pti, poT, et, sc4))
            if pti == 1:
                deferred_oproj = (pti, poT)
            prev_oproj = None
        if ti + 1 < ST and ti + 1 not in st_tiles:
            nxt = load_st(ti + 1)
            st_tiles[ti + 1] = nxt
            if ti + 1 == 1:
                # remaining st0 groups first (attention ti0 needs them)
                side.extend([(v_group, 0, sc, ctx0) for sc in (1, 2, 3)])
                side.extend([(qk_group, 0, cc, w, ctx0)
                             for cc in (1, 2, 3) for w in ("q", "k")])
            if ti == 0:
                # q projections of st1 (must complete before ti1)
                side.extend([(qk_group, ti + 1, cc, "q", nxt)
                             for cc in range(CC)])
        if ti + 2 < ST and ti + 2 not in st_tiles:
            nxt2 = load_st(ti + 2)
            st_tiles[ti + 2] = nxt2
            # q projections of st+2 run in THIS tile's tail (they only need
            # xt(st+2), an input) so ti+1 keeps its slots for k/v + o_proj
            side.extend([(qk_group, ti + 2, cc, "q", nxt2)
                         for cc in range(CC)])

        oT_blks = [[ot_pool.tile([P, P], BF16, name=f"oTb_{pr}_{b}")
                    for b in range(4)] for pr in range(CC)]
        ad = ad_ps.tile([P, 32], F32, name="ad")
        njt = 4 * (ti + 1)
        slots = [(pair, jt) for pair in range(CC) for jt in range(njt)]
        state = {"ad_started": False, "av": None, "av_started": False,
                 "rec": None, "pt": {}, "sc": {}}

        def phase1(slot):
            pair, jt = slot
            r = max(0, (jt - 4 * ti) * P)
            kslc = kT_t[jt // 4][:, pair, (jt % 4) * P:(jt % 4) * P + P]
            scb = sc_ps.tile([P, 1024], F32, name="sc")
            nc.tensor.matmul(
                scb[:, r:512], lhsT=(kslc[0:DK]),
                rhs=(qTs[0:DK, pair, r:512]), start=True, stop=True)
            nc.tensor.matmul(
                scb[:, 512 + r:1024], lhsT=(kslc[DK:P]),
                rhs=(qTs[DK:P, pair, r:512]), start=True, stop=True,
                skip_group_check=True)
            pt = pt_pool.tile([P, 1024], BF16, name="pt")
            if r >= 256:
                # deep-diagonal slots: two exps beat paying for the dead gap
                nc.scalar.activation(
                    pt[:, r:512], scb[:, r:512], AF.Exp, scale=0.125)
                nc.scalar.activation(
                    pt[:, 512 + r:1024], scb[:, 512 + r:1024], AF.Exp,
                    scale=0.125)
            else:
                # one bank-spanning exp covering both heads' ranges; the gap
                # cols [512:512+r) hold stale psum whose exp is never read
                nc.scalar.activation(
                    pt[:, r:1024], scb[:, r:1024], AF.Exp, scale=0.125)
            state["sc"][slot] = scb
            if jt >= 4 * ti:
                nc.vector.tensor_tensor(
                    pt[:, r:r + P], pt[:, r:r + P], msk_sb[:], OP.mult)
                nc.vector.tensor_tensor(
                    pt[:, 512 + r:512 + r + P], pt[:, 512 + r:512 + r + P],
                    msk_sb[:], OP.mult)
            state["pt"][slot] = pt

        def phase2(slot):
            pair, jt = slot
            pt = state["pt"].pop(slot)
            state["sc"].pop(slot)
            if jt == 0:
                state["av"] = av_ps.tile([P, 512], F32, name="av")
                state["av_started"] = False
                state["rec"] = rec_pool.tile([P, 8], F32, name="rec")
            av, rec = state["av"], state["rec"]
            ib0 = max(0, jt - 4 * ti)
            # h0 AV matmuls first (they only wait on the h0 exp), then h1
            for hh in range(2):
                for ib in range(ib0, 4):
                    last = (jt == 4 * ti + ib)
                    h = 2 * pair + hh
                    lhs = (pt[:, ib * P:ib * P + P] if hh == 0
                           else pt[:, 512 + ib * P:512 + ib * P + P])
                    nc.tensor.matmul(
                        av[:, ib * P + hh * DK:ib * P + hh * DK + DK],
                        lhsT=(lhs), rhs=(vv_t[jt][:, h, :]),
                        start=(not state["av_started"]), stop=last,
                        skip_group_check=True)
                    state["av_started"] = True
                    dc = pair * 8 + ib * 2 + hh
                    nc.tensor.matmul(
                        ad[:, dc:dc + 1], lhsT=(lhs), rhs=(ones_sb[:]),
                        start=(not state["ad_started"]), stop=last,
                        skip_group_check=True)
                    state["ad_started"] = True
            ibd = jt - 4 * ti
            if 0 <= ibd <= 3:
                ib = ibd
                # drain (pair, ib): per-partition normalize, PE transpose,
                # copy out to the SBUF lhsT tile for o_proj
                c2 = pair * 8 + ib * 2
                nc.vector.reciprocal(
                    rec[:, ib * 2:ib * 2 + 2], ad[:, c2:c2 + 2])
                o_n = on_pool.tile([P, P], BF16, name="o_n")
                nc.vector.tensor_scalar(
                    o_n[:, 0:DK],
                    av[:, ib * P:ib * P + DK],
                    rec[:, ib * 2:ib * 2 + 1], None, OP.mult)
                nc.vector.tensor_scalar(
                    o_n[:, DK:2 * DK],
                    av[:, ib * P + DK:ib * P + 2 * DK],
                    rec[:, ib * 2 + 1:ib * 2 + 2], None, OP.mult)
                trp = tr_ps.tile([P, P], BF16, name="tr")
                nc.tensor.matmul(
                    trp[:], lhsT=(o_n[:]), rhs=(id_sb[:]),
                    is_transpose=True, start=True, stop=True,
                    skip_group_check=True)
                nc.vector.tensor_copy(oT_blks[pair][ib][:], trp[:])

        # depth-2 software pipeline: sc/exp of slot i+2 issue before the AV
        # block of slot i, so exp latency hides behind PE work
        # ti3's own o_proj for i-block column sc4 only needs pair3's ib=sc4
        # drain (at slot 60+sc4); pull the early columns into the last slots
        tail_oproj = deque()
        if ti == ST - 1:
            for sc4 in range(4):
                for et in range(2):
                    tail_oproj.append((oproj_group, ti, oT_blks, et, sc4))

        nslots = len(slots)
        side_n = len(side)
        popped = 0
        for i in range(nslots):
            phase1(slots[i])
            # side work lands between the scores/exp issue and the
            # exp-blocked AV block (the PE's 4-deep wait queue means work
            # emitted after phase2 can't bypass the stalled AV matmuls),
            # paced evenly across the tile's slots
            want = side_n * (i + 1) // nslots  # floor: keep fill for late slots
            while popped < want and side:
                g = side.popleft()
                g[0](*g[1:])
                popped += 1
            phase2(slots[i])
            if ti == ST - 1 and i >= 61 and tail_oproj:
                for _ in range(2):
                    if tail_oproj:
                        g = tail_oproj.popleft()
                        g[0](*g[1:])
        while side:
            g = side.popleft()
            g[0](*g[1:])
        prev_oproj = (ti, oT_blks)

    # final o_proj (whatever ti3 didn't absorb): alternate between the pp
    # bank and the now-idle scores banks so the tail pipelines
    use_sc = False
    while tail_oproj:
        g = tail_oproj.popleft()
        g[0](*g[1:], use_sc_pool=use_sc, tail=True)
        use_sc = not use_sc


def _host_prep(x, Wq, Wk, Wv, Wo, token_positions):
    """Build the 8 per-core input maps (sharding + layout prep only)."""
    x = np.asarray(x, dtype=np.float32)
    Wq = np.asarray(Wq, dtype=np.float32)
    Wk = np.asarray(Wk, dtype=np.float32)
    Wv = np.asarray(Wv, dtype=np.float32)
    Wo = np.asarray(Wo, dtype=np.float32)
    pos = np.asarray(token_positions)

    half = DK // 2
    inv_freq = THETA ** (-np.arange(half, dtype=np.float64) * 2.0 / DK)
    ang = pos.astype(np.float64)[None, :] * inv_freq[:, None]      # [32, S]
    cos32 = np.cos(ang)
    sin32 = np.sin(ang)
    cosr = np.empty((P, S), dtype=np.float32)
    sinr = np.empty((P, S), dtype=np.float32)
    for p in range(P):
        ip = p % DK
        i = ip % half
        cosr[p] = cos32[i]
        sinr[p] = (-sin32[i]) if ip < half else sin32[i]
    cosr = cosr.astype(ml_dtypes.bfloat16)
    sinr = sinr.astype(ml_dtypes.bfloat16)

    # de-interleave permutation within each head: [evens | odds]
    perm = np.concatenate([np.arange(0, DK, 2), np.arange(1, DK, 2)])

    msk = np.triu(np.ones((P, P), dtype=np.float32)).astype(ml_dtypes.bfloat16)
    ones1 = np.ones((P, 1), dtype=np.float32).astype(ml_dtypes.bfloat16)
    ident = np.eye(P, dtype=np.float32).astype(ml_dtypes.bfloat16)

    WqT = Wq.T  # [d_in, e_out]
    WkT = Wk.T
    WvT = Wv.T
    WoT = Wo.T  # [e_in, d_out]

    in_maps = []
    for core in range(N_CORES):
        b, g = core // HG, core % HG
        cols = np.concatenate(
            [g * CL + h * DK + perm for h in range(H_LOC)])
        plain = slice(g * CL, (g + 1) * CL)
        in_maps.append({
            "xT": np.ascontiguousarray(x[b].T).astype(ml_dtypes.bfloat16),
            "wq": np.ascontiguousarray(WqT[:, cols]).astype(ml_dtypes.bfloat16),
            "wk": np.ascontiguousarray(WkT[:, cols]).astype(ml_dtypes.bfloat16),
            "wv": np.ascontiguousarray(WvT[:, plain]).astype(ml_dtypes.bfloat16),
            "wo": np.ascontiguousarray(WoT[plain, :]).astype(ml_dtypes.bfloat16),
            "cosr": cosr,
            "sinr": sinr,
            "msk": msk,
            "ones1": ones1,
            "ident": ident,
        })
    return in_maps


def kernel(x, Wq, Wk, Wv, Wo, token_positions, _trace=False):
    global LAST_RESULT
    if "nc" not in _CACHE:
        _CACHE["nc"] = _build_program()
    nc = _CACHE["nc"]

    in_maps = _host_prep(x, Wq, Wk, Wv, Wo, token_positions)
    res = run_bass_kernel_spmd(nc, in_maps, core_ids=list(range(N_CORES)),
                               trace=_trace)
    LAST_RESULT = res
    outs = [r["out"] for r in res.results]
    final = np.empty((B, S, D), dtype=np.float32)
    for b in range(B):
        final[b] = outs[b * HG]
        for g in range(1, HG):
            final[b] += outs[b * HG + g]
    return final
